# Initial kernel scaffold
#
"""BitLinear multi-head attention on 8 trn2 NeuronCores.

Sharding: core c handles batch b=c//4 and head group g=c%4 (heads 4g..4g+3).
Stages:
  0. dummy collective at t=0 absorbs the first-collective barrier under the
     input loads + LN stats of the core's 512-row shard of q/k/v
  1. AllReduce(max) of the 3 global absmax scalars
  2. per tensor: quantize to int8-valued bf16, PE-transpose, AllGather
     (three pipelined collectives so projections overlap the later gathers)
  3. projections qpT/kpT (transposed, integer-exact bf16 matmuls) and vp
     (+ bf16 shadow copies qpA/kpA for the max-estimate pass)
  4. attention blocks (h inner, qb outer), software-pipelined; per block the
     PE runs dtype-uniform batches to avoid mode-switch stalls:
       [16 f32r QK matmuls (pass B, paired into [128,1024] PSUM) + 8 exps]
       [16 bf16 att@V matmuls] [16 bf16 pass-A matmuls of the NEXT block]
     pass A estimates the per-query max in bf16 (exp arg stays <= ~+7, safe);
     DVE negated-max reduces write -M via DMA into qpT's augmented row; pass
     B's ones-row/-M-row trick yields exact logits-minus-max; the ones column
     of vps gives the softmax denominator.  After each qb group finishes, its
     x slice is AllGathered (hidden under the next group's compute) and its
     LN stats are emitted one group deferred.
  5. tail: AR2 (global absmax), streamed re-quantization of x, output
     projection (feature-sharded); host assembles the final tensor.
"""
import numpy as np

B, S, D, H, DK = 2, 2048, 1024, 16, 64
SH = 512            # rows per core shard (stage 0)
P = 128
MAGIC = 12582912.0  # 1.5 * 2**23, forces RNE-round-to-int for |x| < 2**22
LN_EPS = 1e-5

_COMPILED = None
DEBUG = False


def _build():
    import concourse.tile as tile
    from concourse import bacc, mybir

    f32 = mybir.dt.float32
    f32r = mybir.dt.float32r
    bf16 = mybir.dt.bfloat16
    EXP = mybir.ActivationFunctionType.Exp
    CPY = mybir.ActivationFunctionType.Copy
    IDN = mybir.ActivationFunctionType.Identity
    SQ = mybir.ActivationFunctionType.Square
    SQRT = mybir.ActivationFunctionType.Sqrt
    ALU = mybir.AluOpType
    AXX = mybir.AxisListType.X

    nc = bacc.Bacc("TRN2", target_bir_lowering=False, debug=False,
                   enable_asserts=False, num_devices=8)

    x_sh = [nc.dram_tensor(f"x{t}", [SH, D], f32, kind="ExternalInput").ap()
            for t in range(3)]
    w_t = [nc.dram_tensor(f"w{t}", [D, 256], bf16, kind="ExternalInput").ap()
           for t in range(3)]                       # sign(w)^T o-slices, bf16
    w0_t = nc.dram_tensor("wo", [D, 256], bf16, kind="ExternalInput").ap()
    id_bf = nc.dram_tensor("idbf", [P, P], bf16, kind="ExternalInput").ap()
    id_f = nc.dram_tensor("idf", [P, P], f32, kind="ExternalInput").ap()
    ones_row = nc.dram_tensor("onesrow", [1, S], f32r, kind="ExternalInput").ap()
    betas = nc.dram_tensor("betas", [1, 8], f32, kind="ExternalInput").ap()
    y = nc.dram_tensor("y", [256, S], f32, kind="ExternalOutput").ap()
    if DEBUG:
        dbg_qpa = nc.dram_tensor("dbg_qpa", [64, 256], bf16, kind="ExternalOutput").ap()
        dbg_kpa = nc.dram_tensor("dbg_kpa", [64, 256], bf16, kind="ExternalOutput").ap()
        dbg_qpt = nc.dram_tensor("dbg_qpt", [65, 256], f32r, kind="ExternalOutput").ap()
        dbg_kpt = nc.dram_tensor("dbg_kpt", [65, 256], f32r, kind="ExternalOutput").ap()
        dbg_vps = nc.dram_tensor("dbg_vps", [P, 260], bf16, kind="ExternalOutput").ap()
        dbg_xsb = nc.dram_tensor("dbg_xsb", [P, 256], f32, kind="ExternalOutput").ap()

    groups8 = [list(range(8))]
    groups4 = [[0, 1, 2, 3], [4, 5, 6, 7]]

    with tile.TileContext(nc) as tc:
        with tc.tile_pool(name="dram", bufs=1, space="DRAM") as dram, \
             tc.tile_pool(name="persist", bufs=1) as per:

            # ---- always-live SBUF ----
            xsb = [per.tile([P, 256], f32, name=f"xsb{i}", tag=f"xsb{i}") for i in range(16)]
            Gb = per.tile([P, 8], f32, name="Gb", tag="Gb")
            Cq = per.tile([P, 1], f32, name="Cq", tag="Cq")
            iCv = per.tile([P, 1], f32, name="iCv", tag="iCv")
            idb = per.tile([P, P], bf16, name="idb", tag="idb")
            idf = per.tile([P, P], f32, name="idf", tag="idf")
            beta_sb = per.tile([1, 8], f32, name="beta_sb", tag="beta_sb")
            ones1 = per.tile([1, P], f32, name="ones1", tag="ones1")
            ones128 = per.tile([P, 1], f32, name="ones128", tag="ones128")
            wq_sb = [per.tile([P, 256], bf16, name=f"wq{ic}", tag=f"wq{ic}")
                     for ic in range(8)]
            wk_sb = [per.tile([P, 256], bf16, name=f"wk{ic}", tag=f"wk{ic}")
                     for ic in range(8)]
            wv_sb = [per.tile([P, 256], bf16, name=f"wv{ic}", tag=f"wv{ic}")
                     for ic in range(8)]
            w0_sb = [per.tile([P, 256], bf16, name=f"w0{ic}", tag=f"w0{ic}")
                     for ic in range(8)]
            nc.sync.dma_start(idb[:], id_bf)
            nc.sync.dma_start(idf[:], id_f)
            nc.sync.dma_start(beta_sb[:], betas)
            nc.vector.memset(ones1[:], 1.0)
            nc.vector.memset(ones128[:], 1.0)
            for ic in range(8):
                nc.sync.dma_start(wq_sb[ic][:], w_t[0][ic * P:(ic + 1) * P, :])
                nc.sync.dma_start(wk_sb[ic][:], w_t[1][ic * P:(ic + 1) * P, :])
                nc.sync.dma_start(wv_sb[ic][:], w_t[2][ic * P:(ic + 1) * P, :])
                nc.sync.dma_start(w0_sb[ic][:], w0_t[ic * P:(ic + 1) * P, :])

            # DRAM bounce buffers
            ag_in = dram.tile([3 * D, SH], bf16, name="ag_in", tag="ag_in")
            ag_out3 = [dram.tile([4 * D, SH], bf16, name=f"ago{t}", tag=f"ago{t}")
                       for t in range(3)]
            ar1_in = dram.tile([1, 8], f32, name="ar1_in", tag="ar1_in")
            ar1_out = dram.tile([1, 8], f32, name="ar1_out", tag="ar1_out")
            ar2_in = dram.tile([1, 8], f32, name="ar2_in", tag="ar2_in")
            ar2_out = dram.tile([1, 8], f32, name="ar2_out", tag="ar2_out")
            dmy_in = dram.tile([1, 8], f32, name="dmy_in", tag="dmy_in")
            dmy_out = dram.tile([1, 8], f32, name="dmy_out", tag="dmy_out")
            xag_in = dram.tile([S, 256], f32, name="xag_in", tag="xag_in")
            xag_out4 = [dram.tile([4 * SH, 256], f32, name=f"xao{j}", tag=f"xao{j}")
                        for j in range(4)]


            def ln_stats_tile(pool, xtile, scr, am_out, sfx):
                """LN stats for one [P, D] tile. Returns (rstd, negmu) tiles."""
                sm = pool.tile([P, 1], f32, name="sm", tag="sm")
                nmu = pool.tile([P, 1], f32, name=f"nmu{sfx}", tag=f"nmu{sfx}")
                ssq = pool.tile([P, 1], f32, name="ssq", tag="ssq")
                rst = pool.tile([P, 1], f32, name=f"rst{sfx}", tag=f"rst{sfx}")
                mxs = pool.tile([P, 1], f32, name="mxs", tag="mxs")
                r2 = pool.tile([P, 1], f32, name="r2", tag="r2")
                nc.vector.tensor_reduce(sm[:], xtile[:], axis=AXX, op=ALU.add)
                nc.scalar.activation(nmu[:], sm[:], CPY, scale=-1.0 / D)
                nc.scalar.activation(scr[:], xtile[:], SQ, bias=nmu[:],
                                     accum_out=ssq[:])
                var_ = pool.tile([P, 1], f32, name="var_", tag="var_")
                rvar = pool.tile([P, 1], f32, name="rvar", tag="rvar")
                y0 = pool.tile([P, 1], f32, name="y0", tag="y0")
                nr = pool.tile([P, 1], f32, name="nr", tag="nr")
                nc.scalar.activation(var_[:], ssq[:], CPY, scale=1.0 / D,
                                     bias=LN_EPS)
                nc.vector.reciprocal(rvar[:], var_[:])
                nc.scalar.activation(y0[:], rvar[:], SQRT)
                # two Newton steps: rstd = y0*(1.5 - 0.5*var*y0^2)
                nc.vector.tensor_tensor(nr[:], y0[:], y0[:], ALU.mult)
                nc.vector.tensor_tensor(nr[:], nr[:], var_[:], ALU.mult)
                nc.vector.tensor_scalar(nr[:], nr[:], -0.5, 1.5, ALU.mult,
                                        ALU.add)
                nc.vector.tensor_tensor(y0[:], y0[:], nr[:], ALU.mult)
                nc.vector.tensor_tensor(nr[:], y0[:], y0[:], ALU.mult)
                nc.vector.tensor_tensor(nr[:], nr[:], var_[:], ALU.mult)
                nc.vector.tensor_scalar(nr[:], nr[:], -0.5, 1.5, ALU.mult,
                                        ALU.add)
                nc.vector.tensor_tensor(rst[:], y0[:], nr[:], ALU.mult)
                nc.vector.tensor_reduce(mxs[:], scr[:], axis=AXX, op=ALU.max)
                nc.vector.tensor_tensor(r2[:], rst[:], rst[:], ALU.mult)
                # squared row amax: max((x-mu)^2) * rstd^2  (no sqrt here)
                nc.vector.tensor_tensor(am_out, mxs[:], r2[:], ALU.mult)
                return rst, nmu

            def tree_max(pool, pspool, src, ncols, out_row):
                """Partition-axis max over src[:, :ncols] -> out_row[0:1, :ncols]."""
                ptr = pspool.tile([P, P], f32, name="trps", tag="trps")
                nc.tensor.transpose(ptr[0:ncols, :], src[:, :ncols], idf[:])
                a = pool.tile([P, P], f32, name="trA", tag="trA")
                nc.scalar.activation(a[0:ncols, :], ptr[0:ncols, :], CPY)
                m = pool.tile([P, 1], f32, name="trM", tag="trM")
                nc.vector.tensor_reduce(m[0:ncols, :], a[0:ncols, :], axis=AXX,
                                        op=ALU.max)
                nc.sync.dma_start(out_row[0:1, 0:ncols], m[0:ncols, :])

            def rsqrt_nr(pool, gsq_ap, sfx):
                """accurate rsqrt of a [P,1] squared-max -> (1/g, g) tiles."""
                rv_ = pool.tile([P, 1], f32, name=f"rv{sfx}", tag=f"rv{sfx}")
                yy = pool.tile([P, 1], f32, name=f"yy{sfx}", tag=f"yy{sfx}")
                tn = pool.tile([P, 1], f32, name=f"tn{sfx}", tag=f"tn{sfx}")
                ig = pool.tile([P, 1], f32, name=f"ig{sfx}", tag=f"ig{sfx}")
                gl = pool.tile([P, 1], f32, name=f"gl{sfx}", tag=f"gl{sfx}")
                nc.vector.reciprocal(rv_[:], gsq_ap)
                nc.scalar.activation(yy[:], rv_[:], SQRT)
                nc.vector.tensor_tensor(tn[:], yy[:], yy[:], ALU.mult)
                nc.vector.tensor_tensor(tn[:], tn[:], gsq_ap, ALU.mult)
                nc.vector.tensor_scalar(tn[:], tn[:], -0.5, 1.5, ALU.mult,
                                        ALU.add)
                nc.vector.tensor_tensor(ig[:], yy[:], tn[:], ALU.mult)
                nc.vector.tensor_tensor(gl[:], gsq_ap, ig[:], ALU.mult)
                return ig, gl

            def quant_tile(pool, pspool, xtile, rst, nmu, s128, dst_tiles, sfx):
                """Quantize one [P, D] tile -> 8 transposed [P, P] bf16 writes."""
                sc_ = pool.tile([P, 1], f32, name="sc_", tag="sc_")
                bp = pool.tile([P, 1], f32, name="bp", tag="bp")
                nc.vector.tensor_tensor(sc_[:], rst[:], s128[:], ALU.mult)
                nc.vector.tensor_tensor(bp[:], nmu[:], sc_[:], ALU.mult)
                # NOTE: MAGIC must NOT fold into the ACT bias — ACT's fused
                # multiply-add rounds once, flipping ints vs the reference's
                # two-step f32 rounding (verified on HW).
                t1a = pool.tile([P, D], f32, name="t1a", tag="t1a")
                t2 = pool.tile([P, D], f32, name="t2", tag="t2")
                msk = pool.tile([P, D], f32, name="msk", tag="msk")
                xqb = pool.tile([P, D], bf16, name="xqb", tag="xqb")
                nc.scalar.activation(t1a[:], xtile[:], IDN, scale=sc_[:], bias=bp[:])
                nc.scalar.activation(t2[:], t1a[:], CPY, bias=MAGIC)
                nc.vector.tensor_scalar(msk[:], t2[:], MAGIC + 127.5, 256.0,
                                        ALU.is_ge, ALU.mult)
                nc.vector.scalar_tensor_tensor(xqb[:], t2[:], -MAGIC, msk[:],
                                               ALU.add, ALU.subtract)
                pstr = pspool.tile([P, D], bf16, name="pstr", tag="pstr",
                                   bufs=2)
                for ic in range(8):
                    nc.tensor.transpose(pstr[:, ic * P:(ic + 1) * P],
                                        xqb[:, ic * P:(ic + 1) * P],
                                        idb[:])
                if isinstance(dst_tiles, list):
                    # DRAM chunk targets: one wide PSUM->SBUF copy, then DMAs
                    stg = pool.tile([P, D], bf16, name="qstg", tag="qstg")
                    nc.vector.tensor_copy(stg[:], pstr[:])
                    for ic in range(8):
                        nc.sync.dma_start(dst_tiles[ic],
                                          stg[:, ic * P:(ic + 1) * P])
                else:
                    # one wide SBUF target: single DVE copy
                    nc.vector.tensor_copy(dst_tiles, pstr[:])

            Am2 = per.tile([P, 16], f32, name="Am2", tag="Am2")
            scr2 = per.tile([P, D], f32, name="scr2", tag="scr2")
            rst2 = [per.tile([P, 1], f32, name=f"rk{rc}", tag=f"rk{rc}")
                    for rc in range(16)]
            nmu2 = [per.tile([P, 1], f32, name=f"nk{rc}", tag=f"nk{rc}")
                    for rc in range(16)]

            def emit_x_stats(qb, pool):
                """deferred stage-5 LN stats for the 4 row-chunks of qb."""
                for rc in range(qb * 4, qb * 4 + 4):
                    xf = pool.tile([P, D], f32, name="xf", tag="xf")
                    for j in range(4):
                        nc.sync.dma_start(
                            xf[:, 256 * j:256 * (j + 1)],
                            xag_out4[qb][SH * j + (rc % 4) * P:
                                         SH * j + (rc % 4 + 1) * P, :])
                    r_, n_ = ln_stats_tile(pool, xf, scr2,
                                           Am2[:, rc:rc + 1], "s5")
                    nc.vector.tensor_copy(rst2[rc][:], r_[:])
                    nc.vector.tensor_copy(nmu2[rc][:], n_[:])

            # ================= stage 0-2: stats, AR, quantize, AGs ===========
            with tc.tile_pool(name="xpool", bufs=1) as xpool, \
                 tc.tile_pool(name="spool", bufs=4) as spool, \
                 tc.tile_pool(name="qpool", bufs=3) as qpool, \
                 tc.tile_pool(name="pst0", bufs=2, space="PSUM") as pst0:
                scr = spool.tile([P, D], f32, name="scr", tag="scr")
                Am = xpool.tile([P, 16], f32, name="Am", tag="Am")
                xt_all, rst_all, nmu_all = [], [], []
                for t in range(3):
                    xts = []
                    for rc in range(4):
                        xt = xpool.tile([P, D], f32, name=f"x{t}_{rc}", tag=f"x{t}_{rc}")
                        nc.sync.dma_start(xt[:], x_sh[t][rc * P:(rc + 1) * P, :])
                        xts.append(xt)
                    rs, nm = [], []
                    for rc in range(4):
                        r_, n_ = ln_stats_tile(spool, xts[rc], scr,
                                               Am[:, 4 * t + rc:4 * t + rc + 1],
                                               f"s0_{t}_{rc}")
                        rs.append(r_)
                        nm.append(n_)
                    xt_all.append(xts)
                    rst_all.append(rs)
                    nmu_all.append(nm)
                # per-tensor max -> its own tiny AllReduce, so quantization
                # of q starts without waiting for k/v stats
                Am3 = xpool.tile([P, 3], f32, name="Am3", tag="Am3")
                arow = xpool.tile([1, 8], f32, name="arow", tag="arow")
                for t in range(3):
                    nc.vector.tensor_reduce(Am3[:, t:t + 1],
                                            Am[:, 4 * t:4 * t + 4],
                                            axis=AXX, op=ALU.max)
                    stage = xpool.tile([1, 8], f32, name=f"stage{t}",
                                       tag=f"stage{t}")
                    nc.vector.memset(stage[:], 0.0)
                    tree_max(spool, pst0, Am3[:, t:t + 1], 1, stage)
                    nc.sync.dma_start(ar1_in[0:1, t:t + 1], stage[0:1, 0:1])
                    nc.gpsimd.collective_compute(
                        "AllReduce", ALU.max, replica_groups=groups8,
                        ins=[ar1_in[0:1, t:t + 1].opt()],
                        outs=[ar1_out[0:1, t:t + 1].opt()])
                    nc.sync.dma_start(arow[0:1, t:t + 1],
                                      ar1_out[0:1, t:t + 1])
                for t in range(3):
                    psb = pst0.tile([P, 1], f32, name="psb", tag="psb")
                    nc.tensor.matmul(psb[:], ones1[:], arow[:, t:t + 1],
                                     start=True, stop=True)
                    nc.vector.tensor_copy(Gb[:, t:t + 1], psb[:])
                    s128 = qpool.tile([P, 1], f32, name="s128", tag="s128")
                    ig_t, _ = rsqrt_nr(qpool, Gb[:, t:t + 1], f"q{t}")
                    nc.vector.tensor_scalar(s128[:], ig_t[:], 128.0, None,
                                            ALU.mult)
                    for rc in range(4):
                        dsts = [ag_in[t * D + ic * P:t * D + (ic + 1) * P,
                                      rc * P:(rc + 1) * P] for ic in range(8)]
                        quant_tile(qpool, pst0, xt_all[t][rc], rst_all[t][rc],
                                   nmu_all[t][rc], s128, dsts, f"q{t}{rc}")
                    nc.gpsimd.collective_compute(
                        "AllGather", ALU.bypass, replica_groups=groups4,
                        ins=[ag_in[t * D:(t + 1) * D, :].opt()],
                        outs=[ag_out3[t].opt()])

                # exp scale C = gq*gk*bq*bk/(128*128*8);  iCv = 128/(gv*bv)
                bb = xpool.tile([P, 8], f32, name="bb", tag="bb")
                psb2 = pst0.tile([P, 8], f32, name="psb2", tag="psb2")
                nc.tensor.matmul(psb2[:], ones1[:], beta_sb[:], start=True,
                                 stop=True)
                nc.vector.tensor_copy(bb[:], psb2[:])
                ct1 = xpool.tile([P, 1], f32, name="ct1", tag="ct1")
                ct2 = xpool.tile([P, 1], f32, name="ct2", tag="ct2")
                _, glq = rsqrt_nr(xpool, Gb[:, 0:1], "lq")
                _, glk = rsqrt_nr(xpool, Gb[:, 1:2], "lk")
                _, glv = rsqrt_nr(xpool, Gb[:, 2:3], "lv")
                nc.vector.tensor_tensor(ct1[:], glq[:], glk[:], ALU.mult)
                nc.vector.tensor_tensor(ct2[:], bb[:, 0:1], bb[:, 1:2], ALU.mult)
                nc.vector.tensor_tensor(ct1[:], ct1[:], ct2[:], ALU.mult)
                nc.vector.tensor_scalar(Cq[:], ct1[:],
                                        1.0 / (128.0 * 128.0 * 8.0), None,
                                        ALU.mult)
                ct3 = xpool.tile([P, 1], f32, name="ct3", tag="ct3")
                nc.vector.tensor_tensor(ct3[:], glv[:], bb[:, 2:3], ALU.mult)
                nc.vector.tensor_scalar(iCv[:], ct3[:], 1.0 / 128.0, None,
                                        ALU.mult)

            # ================= stage 3+4: projections + attention ============
            def agx(t, ic, sc, c0, cn):
                r0 = D * sc + P * ic
                return ag_out3[t][r0:r0 + P, c0:c0 + cn]

            # pools spanning attention + deferred stage-5 stats (s5* stay
            # open past attper for the tail)
            with tc.tile_pool(name="attper", bufs=1) as attper:
                qpT = [attper.tile([65, S], f32r, name=f"qpT{h}", tag=f"qpT{h}") for h in range(4)]
                kpT = [attper.tile([65, S], f32r, name=f"kpT{h}", tag=f"kpT{h}") for h in range(4)]
                qpA = [attper.tile([64, S], bf16, name=f"qpA{h}", tag=f"qpA{h}") for h in range(4)]
                kpA = [attper.tile([64, S], bf16, name=f"kpA{h}", tag=f"kpA{h}") for h in range(4)]
                vps = attper.tile([P, 16 * 260], bf16, name="vps", tag="vps")

                with tc.tile_pool(name="rhsp", bufs=4) as rhsp, \
                     tc.tile_pool(name="psp3", bufs=4, space="PSUM") as psp3:
                    for h in range(4):
                        nc.sync.dma_start(kpT[h][64:65, :], ones_row)
                    for t, wsb, dstT, dstA in ((0, wq_sb, qpT, qpA),
                                               (1, wk_sb, kpT, kpA)):
                        for sc in range(4):
                            rhs = [rhsp.tile([P, SH], bf16, name=f"rh{ic % 4}", tag=f"rh{ic % 4}")
                                   for ic in range(8)]
                            for ic in range(8):
                                nc.sync.dma_start(rhs[ic][:],
                                                  agx(t, ic, sc, 0, SH))
                            for oh in range(2):
                                ps = psp3.tile([P, SH], f32, name="ps3", tag="ps3")
                                for ic in range(8):
                                    nc.tensor.matmul(
                                        ps[:], wsb[ic][:, oh * P:(oh + 1) * P],
                                        rhs[ic][:], start=(ic == 0),
                                        stop=(ic == 7))
                                hA, hB = oh * 2, oh * 2 + 1
                                nc.scalar.activation(
                                    dstT[hA][0:64, sc * SH:(sc + 1) * SH],
                                    ps[0:64, :], CPY)
                                nc.vector.tensor_copy(
                                    dstA[hA][0:64, sc * SH:(sc + 1) * SH],
                                    ps[0:64, :])
                                stg = rhsp.tile([P, SH], f32r, name="stg", tag="stg")
                                stgb = rhsp.tile([P, SH], bf16, name="stgb", tag="stgb")
                                nc.scalar.activation(stg[64:P, :],
                                                     ps[64:P, :], CPY)
                                nc.vector.tensor_copy(stgb[64:P, :], ps[64:P, :])
                                nc.sync.dma_start(
                                    dstT[hB][0:64, sc * SH:(sc + 1) * SH],
                                    stg[64:P, :])
                                nc.sync.dma_start(
                                    dstA[hB][0:64, sc * SH:(sc + 1) * SH],
                                    stgb[64:P, :])
                    for sc4 in range(4):
                      vstr = [rhsp.tile([P, SH], bf16, name=f"vs{ic % 4}",
                                        tag=f"vs{ic % 4}") for ic in range(8)]
                      for ic in range(8):
                          nc.sync.dma_start(vstr[ic][:], agx(2, ic, sc4, 0, SH))
                      for qc in range(4):
                        kc = sc4 * 4 + qc
                        ps = psp3.tile([P, 256], f32, name="psv", tag="psv")
                        for ic in range(8):
                            nc.tensor.matmul(ps[:],
                                             vstr[ic][:, qc * P:(qc + 1) * P],
                                             wv_sb[ic][:],
                                             start=(ic == 0), stop=(ic == 7))
                        for h in range(4):
                            nc.scalar.activation(
                                vps[:, 260 * kc + 65 * h:260 * kc + 65 * h + 64],
                                ps[:, 64 * h:64 * h + 64], CPY)
                            nc.scalar.activation(
                                vps[:, 260 * kc + 65 * h + 64:
                                    260 * kc + 65 * h + 65],
                                ones128[:], CPY)

                # ---- attention: qb-outer blocks, dtype-uniform PE batches ----
                with tc.tile_pool(name="mpool", bufs=2) as mpool, \
                     tc.tile_pool(name="ptp", bufs=8) as ptp, \
                     tc.tile_pool(name="xtsb", bufs=2) as xtsb, \
                     tc.tile_pool(name="psA", bufs=2, space="PSUM") as psA, \
                     tc.tile_pool(name="psB", bufs=2, space="PSUM") as psB, \
                     tc.tile_pool(name="psX", bufs=1, space="PSUM") as psX, \
                     tc.tile_pool(name="psT", bufs=1, space="PSUM") as psT:

                    blocks = [(h, qb) for qb in range(4) for h in range(4)]

                    mp_tiles = {}
                    xps_tiles = {}

                    def new_mp(blk):
                        for qc in range(4):
                            mp_tiles[blk + (qc,)] = mpool.tile(
                                [P, 4], f32, name="Mp", tag=f"Mp{qc % 2}")

                    def new_xps(blk):
                        xps_tiles[blk] = psX.tile([65, SH], f32,
                                                  name="xps", tag="xps")

                    def emit_passA_mm(h, qb, step):
                        """one bf16 QK chunk [128 queries x 512 keys] + DVE max."""
                        q0 = qb * SH
                        qc, kb = step // 4, step % 4
                        pa = psA.tile([P, SH], f32, name="pa", tag="pa")
                        nc.tensor.matmul(
                            pa[:],
                            qpA[h][:, q0 + qc * P:q0 + (qc + 1) * P],
                            kpA[h][:, kb * SH:(kb + 1) * SH],
                            start=True, stop=True)
                        Mp = mp_tiles[(h, qb, qc)]
                        nc.vector.tensor_reduce(Mp[:, kb:kb + 1], pa[:],
                                                axis=AXX, op=ALU.max)
                        if kb == 3:
                            ngm = mpool.tile([P, 1], f32r, name="ngm", tag="ngm")
                            nc.vector.tensor_reduce(ngm[:], Mp[:], axis=AXX,
                                                    op=ALU.max, negate=True)
                            nc.sync.dma_start(
                                qpT[h][64:65, q0 + qc * P:q0 + (qc + 1) * P],
                                ngm[:])
                            mp_tiles.pop((h, qb, qc))

                    pb_cur = [None]
                    pT_list = []

                    def emit_B(h, qb, kc):
                        """one f32r QK matmul into a [128,1024] PSUM half;
                        exp fires per completed pair."""
                        q0 = qb * SH
                        half = kc % 2
                        if half == 0:
                            pb_cur[0] = psB.tile([P, 2 * SH], f32, name="pb",
                                                 tag="pb")
                        pb = pb_cur[0]
                        nc.tensor.matmul(
                            pb[:, half * SH:(half + 1) * SH],
                            kpT[h][:, kc * P:(kc + 1) * P],
                            qpT[h][:, q0:q0 + SH],
                            start=True, stop=True)
                        if half == 1:
                            pT = ptp.tile([P, 2 * SH], bf16, name="pT", tag="pT")
                            nc.scalar.activation(pT[:], pb[:], EXP, scale=Cq[:])
                            pT_list.append(pT)

                    def emit_AV(h, qb, kc):
                        xps = xps_tiles[(h, qb)]
                        pT = pT_list[kc // 2]
                        half = kc % 2
                        nc.tensor.matmul(
                            xps[:],
                            vps[:, 260 * kc + 65 * h:260 * kc + 65 * (h + 1)],
                            pT[:, half * SH:(half + 1) * SH],
                            start=(kc == 0), stop=(kc == 15))

                    def emit_epilogue(h, qb):
                        xps = xps_tiles.pop((h, qb))
                        xt_s = xtsb.tile([65, SH], f32, name="xt_s", tag="xt_s")
                        nc.scalar.activation(xt_s[:], xps[:], CPY)
                        for qc in range(4):
                            ptx = psT.tile([P, 65], f32, name="ptx", tag="ptx")
                            nc.tensor.transpose(
                                ptx[:], xt_s[0:65, qc * P:(qc + 1) * P],
                                idf[0:65, 0:65])
                            rv = mpool.tile([P, 1], f32, name="rv", tag="rv")
                            rv0 = mpool.tile([P, 1], f32, name="rv0", tag="rv0")
                            nc.vector.reciprocal(rv0[:], ptx[:, 64:65])
                            nc.vector.tensor_tensor(rv[:], rv0[:], iCv[:],
                                                    ALU.mult)
                            nc.vector.tensor_scalar(
                                xsb[qb * 4 + qc][:, 64 * h:64 * (h + 1)],
                                ptx[:, 0:64], rv[:], None, ALU.mult)

                    # prologue: pass A for block 0
                    new_mp(blocks[0])
                    for step in range(16):
                        emit_passA_mm(*blocks[0], step)

                    for i, blk in enumerate(blocks):
                        h, qb = blk
                        nxt = blocks[i + 1] if i + 1 < len(blocks) else None
                        new_xps(blk)
                        if nxt is not None:
                            new_mp(nxt)
                        pT_list.clear()
                        # fine interleave: pass A of next block + B/exp of this
                        # one; AV trails B by two steps so the PE never waits
                        # on the ACT exp (sustains the 2.4GHz p-state)
                        for kc in range(16):
                            if nxt is not None:
                                emit_passA_mm(*nxt, kc)
                            emit_B(h, qb, kc)
                            if kc >= 2:
                                emit_AV(h, qb, kc - 2)
                        emit_AV(h, qb, 14)
                        emit_AV(h, qb, 15)
                        emit_epilogue(h, qb)
                        if h == 3:
                            # qb group finished: ship x slice, launch its AG
                            for qc in range(4):
                                nc.sync.dma_start(
                                    xag_in[qb * SH + qc * P:
                                           qb * SH + (qc + 1) * P, :],
                                    xsb[qb * 4 + qc][:])
                            nc.gpsimd.collective_compute(
                                "AllGather", ALU.bypass, replica_groups=groups4,
                                ins=[xag_in[qb * SH:(qb + 1) * SH, :].opt()],
                                outs=[xag_out4[qb].opt()])
                        if h == 1 and qb >= 1:
                            # stats of the previous group, 2 blocks after its
                            # AG launched (landed by now; no HOL blocking)
                            emit_x_stats(qb - 1, mpool)

                if DEBUG:
                    with tc.tile_pool(name="dbgp", bufs=1) as dbgp:
                        nc.sync.dma_start(dbg_qpa, qpA[0][:, 0:256])
                        nc.sync.dma_start(dbg_kpa, kpA[0][:, 0:256])
                        nc.sync.dma_start(dbg_qpt, qpT[0][:, 0:256])
                        nc.sync.dma_start(dbg_kpt, kpT[0][:, 0:256])
                        nc.sync.dma_start(dbg_vps, vps[:, 0:260])
                        nc.sync.dma_start(dbg_xsb, xsb[0][:])

            # ====== stage 5 tail: last stats, AR2, quant, projection =====
            with tc.tile_pool(name="fpool", bufs=1) as fpool, \
                 tc.tile_pool(name="f2pool", bufs=3) as f2pool, \
                 tc.tile_pool(name="psf", bufs=1, space="PSUM") as psf:
                emit_x_stats(3, f2pool)
                xq0T = fpool.tile([P, 16 * D], bf16, name="xq0T", tag="xq0T")
                Am21 = fpool.tile([P, 1], f32, name="Am21", tag="Am21")
                nc.vector.tensor_reduce(Am21[:], Am2[:], axis=AXX, op=ALU.max)
                stage2 = fpool.tile([1, 8], f32, name="stage2", tag="stage2")
                nc.vector.memset(stage2[:], 0.0)
                tree_max(f2pool, psf, Am21, 1, stage2)
                nc.sync.dma_start(ar2_in[:], stage2[:])
                nc.gpsimd.collective_compute(
                    "AllReduce", ALU.max, replica_groups=groups8,
                    ins=[ar2_in.opt()], outs=[ar2_out.opt()])
                arow2 = fpool.tile([1, 8], f32, name="arow2", tag="arow2")
                nc.sync.dma_start(arow2[:], ar2_out[:])
                psb3 = psf.tile([P, 8], f32, name="psb3", tag="psb3")
                nc.tensor.matmul(psb3[:], ones1[:], arow2[:], start=True,
                                 stop=True)
                G2 = fpool.tile([P, 8], f32, name="G2", tag="G2")
                nc.vector.tensor_copy(G2[:], psb3[:])
                s128b = fpool.tile([P, 1], f32, name="s128b", tag="s128b")
                ig0, gl0 = rsqrt_nr(fpool, G2[:, 0:1], "f0")
                nc.vector.tensor_scalar(s128b[:], ig0[:], 128.0, None,
                                        ALU.mult)
                bb2 = fpool.tile([P, 8], f32, name="bb2", tag="bb2")
                psb4 = psf.tile([P, 8], f32, name="psb4", tag="psb3")
                nc.tensor.matmul(psb4[:], ones1[:], beta_sb[:], start=True,
                                 stop=True)
                nc.vector.tensor_copy(bb2[:], psb4[:])
                C0 = fpool.tile([P, 1], f32, name="C0", tag="C0")
                nc.vector.tensor_tensor(C0[:], gl0[:], bb2[:, 3:4], ALU.mult)
                nc.vector.tensor_scalar(C0[:], C0[:], 1.0 / 128.0, None,
                                        ALU.mult)
                for rc in range(16):
                    xf = f2pool.tile([P, D], f32, name="xf2", tag="xf2")
                    qb = rc // 4
                    for j in range(4):
                        nc.sync.dma_start(
                            xf[:, 256 * j:256 * (j + 1)],
                            xag_out4[qb][SH * j + (rc % 4) * P:
                                         SH * j + (rc % 4 + 1) * P, :])
                    quant_tile(f2pool, psf, xf, rst2[rc], nmu2[rc],
                               s128b, xq0T[:, rc * D:(rc + 1) * D], f"f{rc}")
                # flipped output projection: y^T[outs, rows], 512-row chunks
                xq4 = xq0T[:].rearrange("p (rc ic c) -> p rc ic c",
                                        rc=16, ic=8, c=P)
                for oh in range(2):
                    for r4 in range(4):
                        ps = psf.tile([P, SH], f32, name="pso", tag="pso",
                                      bufs=2)
                        for ic in range(8):
                            nc.tensor.matmul(
                                ps[:],
                                w0_sb[ic][:, oh * P:(oh + 1) * P],
                                xq4[:, 4 * r4:4 * (r4 + 1), ic, :],
                                start=(ic == 0), stop=(ic == 7))
                        yt = f2pool.tile([P, SH], f32, name="yt", tag="yt")
                        nc.scalar.activation(yt[:], ps[:], CPY, scale=C0[:])
                        nc.sync.dma_start(
                            y[oh * P:(oh + 1) * P, r4 * SH:(r4 + 1) * SH],
                            yt[:])

    nc.compile()
    return nc


def _prep_host(inputs):
    import ml_dtypes
    bf = ml_dtypes.bfloat16
    ws = [inputs["wq_w"], inputs["wk_w"], inputs["wv_w"], inputs["w0_w"]]
    signs = []
    betas = np.zeros((1, 8), np.float32)
    for i, w in enumerate(ws):
        w64 = np.asarray(w, np.float64)
        signs.append(np.sign(w64 - w64.mean()).astype(np.float32))
        betas[0, i] = np.abs(w64).mean()
    id_bf = np.eye(128, dtype=bf)
    id_f = np.eye(128, dtype=np.float32)
    ones_row = np.ones((1, S), np.float32)
    qf = np.asarray(inputs["q"], np.float32).reshape(2 * S, D)
    kf = np.asarray(inputs["k"], np.float32).reshape(2 * S, D)
    vf = np.asarray(inputs["v"], np.float32).reshape(2 * S, D)
    in_maps = []
    for c in range(8):
        b, g = c // 4, c % 4
        r0 = b * S + g * SH
        m = {
            "x0": np.ascontiguousarray(qf[r0:r0 + SH]),
            "x1": np.ascontiguousarray(kf[r0:r0 + SH]),
            "x2": np.ascontiguousarray(vf[r0:r0 + SH]),
            "wo": np.ascontiguousarray(
                signs[3].T[:, 256 * g:256 * (g + 1)]).astype(bf),
            "idbf": id_bf, "idf": id_f,
            "onesrow": ones_row, "betas": betas,
        }
        for t in range(3):
            m[f"w{t}"] = np.ascontiguousarray(
                signs[t].T[:, 256 * g:256 * (g + 1)]).astype(bf)
        in_maps.append(m)
    return in_maps


def _run(inputs, trace=False):
    global _COMPILED
    from concourse import bass_utils
    if _COMPILED is None:
        _COMPILED = _build()
    nc = _COMPILED
    in_maps = _prep_host(inputs)
    res = bass_utils.run_bass_kernel_spmd(nc, in_maps, core_ids=list(range(8)),
                                          trace=trace)
    out = np.zeros((B, S, D), np.float32)
    for c in range(8):
        b, g = c // 4, c % 4
        out[b, :, 256 * g:256 * (g + 1)] = res.results[c]["y"].T
    return out, res


def kernel(**inputs):
    mask = np.asarray(inputs["mask"])
    if not (mask == 1).all():
        return _numpy_fallback(**inputs)
    out, _ = _run(inputs, trace=False)
    return out


def _numpy_fallback(q, k, v, mask, wq_w, wk_w, wv_w, w0_w):
    f = np.float32

    def ln(x):
        mu = x.mean(-1, keepdims=True, dtype=f)
        var = np.mean((x - mu) ** 2, -1, keepdims=True, dtype=f)
        return ((x - mu) / np.sqrt(var + f(LN_EPS))).astype(f)

    def bitlin(x, w):
        xn = ln(np.asarray(x, f))
        mx = np.abs(xn).max()
        xq = np.round(xn * (f(128.0) / mx)).astype(f)
        xq = (np.mod(xq + 128.0, 256.0) - 128.0).astype(f)
        wq = np.sign(w - w.mean(dtype=f)).astype(f)
        beta = np.abs(w).mean(dtype=f)
        return ((xq @ wq.T) * f(mx / 128 * beta)).astype(f)

    qp = bitlin(q, wq_w).reshape(B, S, H, DK).transpose(0, 2, 1, 3)
    kp = bitlin(k, wk_w).reshape(B, S, H, DK).transpose(0, 2, 1, 3)
    vp = bitlin(v, wv_w).reshape(B, S, H, DK).transpose(0, 2, 1, 3)
    out = np.zeros((B, H, S, DK), f)
    mask = np.asarray(mask)
    for b in range(B):
        for h in range(H):
            att = (qp[b, h] @ kp[b, h].T) / f(np.sqrt(DK))
            att = np.where(mask[b] == 0, f(-1e9), att).astype(f)
            att = att - att.max(-1, keepdims=True)
            e = np.exp(att)
            p = e / e.sum(-1, keepdims=True)
            out[b, h] = p @ vp[b, h]
    x = out.transpose(0, 2, 1, 3).reshape(B, S, H * DK)
    return bitlin(x, w0_w)



# revision 33
# speedup vs baseline: 1.0290x; 1.0290x over previous
"""BitLinear multi-head attention on 8 trn2 NeuronCores.

Sharding: core c handles batch b=c//4 and head group g=c%4 (heads 4g..4g+3).
Stages:
  0. dummy collective at t=0 absorbs the first-collective barrier under the
     input loads + LN stats of the core's 512-row shard of q/k/v
  1. AllReduce(max) of the 3 global absmax scalars
  2. per tensor: quantize to int8-valued bf16, PE-transpose, AllGather
     (three pipelined collectives so projections overlap the later gathers)
  3. projections qpT/kpT (transposed, integer-exact bf16 matmuls) and vp
     (+ bf16 shadow copies qpA/kpA for the max-estimate pass)
  4. attention blocks (h inner, qb outer), software-pipelined; per block the
     PE runs dtype-uniform batches to avoid mode-switch stalls:
       [16 f32r QK matmuls (pass B, paired into [128,1024] PSUM) + 8 exps]
       [16 bf16 att@V matmuls] [16 bf16 pass-A matmuls of the NEXT block]
     pass A estimates the per-query max in bf16 (exp arg stays <= ~+7, safe);
     DVE negated-max reduces write -M via DMA into qpT's augmented row; pass
     B's ones-row/-M-row trick yields exact logits-minus-max; the ones column
     of vps gives the softmax denominator.  After each qb group finishes, its
     x slice is AllGathered (hidden under the next group's compute) and its
     LN stats are emitted one group deferred.
  5. tail: AR2 (global absmax), streamed re-quantization of x, output
     projection (feature-sharded); host assembles the final tensor.
"""
import numpy as np

B, S, D, H, DK = 2, 2048, 1024, 16, 64
SH = 512            # rows per core shard (stage 0)
P = 128
MAGIC = 12582912.0  # 1.5 * 2**23, forces RNE-round-to-int for |x| < 2**22
LN_EPS = 1e-5

_COMPILED = None
DEBUG = False


def _build():
    import concourse.tile as tile
    from concourse import bacc, mybir

    f32 = mybir.dt.float32
    f32r = mybir.dt.float32r
    bf16 = mybir.dt.bfloat16
    EXP = mybir.ActivationFunctionType.Exp
    CPY = mybir.ActivationFunctionType.Copy
    IDN = mybir.ActivationFunctionType.Identity
    SQ = mybir.ActivationFunctionType.Square
    SQRT = mybir.ActivationFunctionType.Sqrt
    ALU = mybir.AluOpType
    AXX = mybir.AxisListType.X

    nc = bacc.Bacc("TRN2", target_bir_lowering=False, debug=False,
                   enable_asserts=False, num_devices=8)

    x_sh = [nc.dram_tensor(f"x{t}", [SH, D], f32, kind="ExternalInput").ap()
            for t in range(3)]
    w_t = [nc.dram_tensor(f"w{t}", [D, 256], bf16, kind="ExternalInput").ap()
           for t in range(3)]                       # sign(w)^T o-slices, bf16
    w0_t = nc.dram_tensor("wo", [D, 256], bf16, kind="ExternalInput").ap()
    id_bf = nc.dram_tensor("idbf", [P, P], bf16, kind="ExternalInput").ap()
    id_f = nc.dram_tensor("idf", [P, P], f32, kind="ExternalInput").ap()
    ones_row = nc.dram_tensor("onesrow", [1, S], f32r, kind="ExternalInput").ap()
    betas = nc.dram_tensor("betas", [1, 8], f32, kind="ExternalInput").ap()
    y = nc.dram_tensor("y", [256, S], f32, kind="ExternalOutput").ap()
    if DEBUG:
        dbg_qpa = nc.dram_tensor("dbg_qpa", [64, 256], bf16, kind="ExternalOutput").ap()
        dbg_kpa = nc.dram_tensor("dbg_kpa", [64, 256], bf16, kind="ExternalOutput").ap()
        dbg_qpt = nc.dram_tensor("dbg_qpt", [65, 256], f32r, kind="ExternalOutput").ap()
        dbg_kpt = nc.dram_tensor("dbg_kpt", [65, 256], f32r, kind="ExternalOutput").ap()
        dbg_vps = nc.dram_tensor("dbg_vps", [P, 260], bf16, kind="ExternalOutput").ap()
        dbg_xsb = nc.dram_tensor("dbg_xsb", [P, 256], f32, kind="ExternalOutput").ap()

    groups8 = [list(range(8))]
    groups4 = [[0, 1, 2, 3], [4, 5, 6, 7]]

    with tile.TileContext(nc) as tc:
        with tc.tile_pool(name="dram", bufs=1, space="DRAM") as dram, \
             tc.tile_pool(name="persist", bufs=1) as per:

            # ---- always-live SBUF ----
            xsb = [per.tile([P, 256], bf16, name=f"xsb{i}", tag=f"xsb{i}") for i in range(16)]
            Gb = per.tile([P, 8], f32, name="Gb", tag="Gb")
            Cq = per.tile([P, 1], f32, name="Cq", tag="Cq")
            iCv = per.tile([P, 1], f32, name="iCv", tag="iCv")
            idb = per.tile([P, P], bf16, name="idb", tag="idb")
            idf = per.tile([P, P], f32, name="idf", tag="idf")
            beta_sb = per.tile([1, 8], f32, name="beta_sb", tag="beta_sb")
            ones1 = per.tile([1, P], f32, name="ones1", tag="ones1")
            ones128 = per.tile([P, 1], f32, name="ones128", tag="ones128")
            wq_sb = [per.tile([P, 256], bf16, name=f"wq{ic}", tag=f"wq{ic}")
                     for ic in range(8)]
            wk_sb = [per.tile([P, 256], bf16, name=f"wk{ic}", tag=f"wk{ic}")
                     for ic in range(8)]
            wv_sb = [per.tile([P, 256], bf16, name=f"wv{ic}", tag=f"wv{ic}")
                     for ic in range(8)]
            w0_sb = [per.tile([P, 256], bf16, name=f"w0{ic}", tag=f"w0{ic}")
                     for ic in range(8)]
            nc.sync.dma_start(idb[:], id_bf)
            nc.sync.dma_start(idf[:], id_f)
            nc.sync.dma_start(beta_sb[:], betas)
            nc.vector.memset(ones1[:], 1.0)
            nc.vector.memset(ones128[:], 1.0)

            def emit_weight_loads():
                # emitted after the x-shard loads: the sync queue drains in
                # order, and stats must not wait behind 8MB of weights
                for ic in range(8):
                    nc.sync.dma_start(wq_sb[ic][:],
                                      w_t[0][ic * P:(ic + 1) * P, :])
                    nc.sync.dma_start(wk_sb[ic][:],
                                      w_t[1][ic * P:(ic + 1) * P, :])
                    nc.sync.dma_start(wv_sb[ic][:],
                                      w_t[2][ic * P:(ic + 1) * P, :])
                    nc.sync.dma_start(w0_sb[ic][:],
                                      w0_t[ic * P:(ic + 1) * P, :])

            # DRAM bounce buffers
            i8 = mybir.dt.int8
            ag_in = dram.tile([3 * D, SH], i8, name="ag_in", tag="ag_in")
            ag_out3 = [dram.tile([4 * D, SH], i8, name=f"ago{t}", tag=f"ago{t}")
                       for t in range(3)]
            ar1_in = dram.tile([1, 8], f32, name="ar1_in", tag="ar1_in")
            ar1_out = dram.tile([1, 8], f32, name="ar1_out", tag="ar1_out")
            ar2_in = dram.tile([1, 8], f32, name="ar2_in", tag="ar2_in")
            ar2_out = dram.tile([1, 8], f32, name="ar2_out", tag="ar2_out")
            dmy_in = dram.tile([1, 8], f32, name="dmy_in", tag="dmy_in")
            dmy_out = dram.tile([1, 8], f32, name="dmy_out", tag="dmy_out")
            xag_in = dram.tile([S, 256], bf16, name="xag_in", tag="xag_in")
            xag_out4 = [dram.tile([4 * SH, 256], bf16, name=f"xao{j}",
                                  tag=f"xao{j}") for j in range(4)]


            def ln_stats_tile(pool, xtile, scr, am_out, sfx):
                """LN stats for one [P, D] tile. Returns (rstd, negmu) tiles."""
                sm = pool.tile([P, 1], f32, name="sm", tag="sm")
                nmu = pool.tile([P, 1], f32, name=f"nmu{sfx}", tag=f"nmu{sfx}")
                ssq = pool.tile([P, 1], f32, name="ssq", tag="ssq")
                rst = pool.tile([P, 1], f32, name=f"rst{sfx}", tag=f"rst{sfx}")
                mxs = pool.tile([P, 1], f32, name="mxs", tag="mxs")
                r2 = pool.tile([P, 1], f32, name="r2", tag="r2")
                nc.vector.tensor_reduce(sm[:], xtile[:], axis=AXX, op=ALU.add)
                nc.scalar.activation(nmu[:], sm[:], CPY, scale=-1.0 / D)
                nc.scalar.activation(scr[:], xtile[:], SQ, bias=nmu[:],
                                     accum_out=ssq[:])
                var_ = pool.tile([P, 1], f32, name="var_", tag="var_")
                rvar = pool.tile([P, 1], f32, name="rvar", tag="rvar")
                y0 = pool.tile([P, 1], f32, name="y0", tag="y0")
                nr = pool.tile([P, 1], f32, name="nr", tag="nr")
                nc.scalar.activation(var_[:], ssq[:], CPY, scale=1.0 / D,
                                     bias=LN_EPS)
                nc.vector.reciprocal(rvar[:], var_[:])
                nc.scalar.activation(y0[:], rvar[:], SQRT)
                # two Newton steps: rstd = y0*(1.5 - 0.5*var*y0^2)
                nc.vector.tensor_tensor(nr[:], y0[:], y0[:], ALU.mult)
                nc.vector.tensor_tensor(nr[:], nr[:], var_[:], ALU.mult)
                nc.vector.tensor_scalar(nr[:], nr[:], -0.5, 1.5, ALU.mult,
                                        ALU.add)
                nc.vector.tensor_tensor(y0[:], y0[:], nr[:], ALU.mult)
                nc.vector.tensor_tensor(nr[:], y0[:], y0[:], ALU.mult)
                nc.vector.tensor_tensor(nr[:], nr[:], var_[:], ALU.mult)
                nc.vector.tensor_scalar(nr[:], nr[:], -0.5, 1.5, ALU.mult,
                                        ALU.add)
                nc.vector.tensor_tensor(rst[:], y0[:], nr[:], ALU.mult)
                nc.vector.tensor_reduce(mxs[:], scr[:], axis=AXX, op=ALU.max)
                nc.vector.tensor_tensor(r2[:], rst[:], rst[:], ALU.mult)
                # squared row amax: max((x-mu)^2) * rstd^2  (no sqrt here)
                nc.vector.tensor_tensor(am_out, mxs[:], r2[:], ALU.mult)
                return rst, nmu

            def tree_max(pool, pspool, src, ncols, out_row):
                """Partition-axis max over src[:, :ncols] -> out_row[0:1, :ncols]."""
                ptr = pspool.tile([P, P], f32, name="trps", tag="trps")
                nc.tensor.transpose(ptr[0:ncols, :], src[:, :ncols], idf[:])
                a = pool.tile([P, P], f32, name="trA", tag="trA")
                nc.scalar.activation(a[0:ncols, :], ptr[0:ncols, :], CPY)
                m = pool.tile([P, 1], f32, name="trM", tag="trM")
                nc.vector.tensor_reduce(m[0:ncols, :], a[0:ncols, :], axis=AXX,
                                        op=ALU.max)
                nc.sync.dma_start(out_row[0:1, 0:ncols], m[0:ncols, :])

            def rsqrt_nr(pool, gsq_ap, sfx):
                """accurate rsqrt of a [P,1] squared-max -> (1/g, g) tiles."""
                rv_ = pool.tile([P, 1], f32, name=f"rv{sfx}", tag=f"rv{sfx}")
                yy = pool.tile([P, 1], f32, name=f"yy{sfx}", tag=f"yy{sfx}")
                tn = pool.tile([P, 1], f32, name=f"tn{sfx}", tag=f"tn{sfx}")
                ig = pool.tile([P, 1], f32, name=f"ig{sfx}", tag=f"ig{sfx}")
                gl = pool.tile([P, 1], f32, name=f"gl{sfx}", tag=f"gl{sfx}")
                nc.vector.reciprocal(rv_[:], gsq_ap)
                nc.scalar.activation(yy[:], rv_[:], SQRT)
                nc.vector.tensor_tensor(tn[:], yy[:], yy[:], ALU.mult)
                nc.vector.tensor_tensor(tn[:], tn[:], gsq_ap, ALU.mult)
                nc.vector.tensor_scalar(tn[:], tn[:], -0.5, 1.5, ALU.mult,
                                        ALU.add)
                nc.vector.tensor_tensor(ig[:], yy[:], tn[:], ALU.mult)
                nc.vector.tensor_tensor(gl[:], gsq_ap, ig[:], ALU.mult)
                return ig, gl

            def quant_tile(pool, pspool, xtile, rst, nmu, s128, dst_tiles, sfx):
                """Quantize one [P, D] tile -> 8 transposed [P, P] bf16 writes."""
                sc_ = pool.tile([P, 1], f32, name="sc_", tag="sc_")
                bp = pool.tile([P, 1], f32, name="bp", tag="bp")
                nc.vector.tensor_tensor(sc_[:], rst[:], s128[:], ALU.mult)
                nc.vector.tensor_tensor(bp[:], nmu[:], sc_[:], ALU.mult)
                # NOTE: MAGIC must NOT fold into the ACT bias — ACT's fused
                # multiply-add rounds once, flipping ints vs the reference's
                # two-step f32 rounding (verified on HW).
                t1a = pool.tile([P, D], f32, name="t1a", tag="t1a")
                t2 = pool.tile([P, D], f32, name="t2", tag="t2")
                msk = pool.tile([P, D], f32, name="msk", tag="msk")
                xqb = pool.tile([P, D], bf16, name="xqb", tag="xqb")
                nc.scalar.activation(t1a[:], xtile[:], IDN, scale=sc_[:], bias=bp[:])
                nc.scalar.activation(t2[:], t1a[:], CPY, bias=MAGIC)
                nc.vector.tensor_scalar(msk[:], t2[:], MAGIC + 127.5, 256.0,
                                        ALU.is_ge, ALU.mult)
                nc.vector.scalar_tensor_tensor(xqb[:], t2[:], -MAGIC, msk[:],
                                               ALU.add, ALU.subtract)
                pstr = pspool.tile([P, D], bf16, name="pstr", tag="pstr",
                                   bufs=2)
                for ic in range(8):
                    nc.tensor.transpose(pstr[:, ic * P:(ic + 1) * P],
                                        xqb[:, ic * P:(ic + 1) * P],
                                        idb[:])
                if isinstance(dst_tiles, list):
                    # DRAM chunk targets: one wide PSUM->SBUF cast to int8
                    # (halves the AllGather payload), then DMAs
                    stg = pool.tile([P, D], mybir.dt.int8, name="qstg",
                                    tag="qstg")
                    nc.vector.tensor_copy(stg[:], pstr[:])
                    for ic in range(8):
                        nc.sync.dma_start(dst_tiles[ic],
                                          stg[:, ic * P:(ic + 1) * P])
                else:
                    # one wide SBUF target: single DVE copy
                    nc.vector.tensor_copy(dst_tiles, pstr[:])

            Am2 = per.tile([P, 16], f32, name="Am2", tag="Am2")
            scr2 = per.tile([P, D], f32, name="scr2", tag="scr2")
            rst2 = [per.tile([P, 1], f32, name=f"rk{rc}", tag=f"rk{rc}")
                    for rc in range(16)]
            nmu2 = [per.tile([P, 1], f32, name=f"nk{rc}", tag=f"nk{rc}")
                    for rc in range(16)]

            def emit_x_stat_tile(rc, pool):
                """deferred stage-5 LN stats for one 128-row chunk of x."""
                qb = rc // 4
                xf = pool.tile([P, D], bf16, name="xf", tag="xf")
                for j in range(4):
                    # gpsimd SWDGE: a wait on the AG here must not HOL-block
                    # the sync queue that carries the -M row DMAs
                    nc.gpsimd.dma_start(
                        xf[:, 256 * j:256 * (j + 1)],
                        xag_out4[qb][SH * j + (rc % 4) * P:
                                     SH * j + (rc % 4 + 1) * P, :])
                r_, n_ = ln_stats_tile(pool, xf, scr2,
                                       Am2[:, rc:rc + 1], "s5")
                nc.vector.tensor_copy(rst2[rc][:], r_[:])
                nc.vector.tensor_copy(nmu2[rc][:], n_[:])

            # ================= stage 0-2: stats, AR, quantize, AGs ===========
            with tc.tile_pool(name="xpool", bufs=1) as xpool, \
                 tc.tile_pool(name="spool", bufs=4) as spool, \
                 tc.tile_pool(name="qpool", bufs=3) as qpool, \
                 tc.tile_pool(name="pst0", bufs=2, space="PSUM") as pst0:
                scr = spool.tile([P, D], f32, name="scr", tag="scr")
                Am = xpool.tile([P, 16], f32, name="Am", tag="Am")
                xt_all, rst_all, nmu_all = [], [], []
                for t in range(3):
                    xts = []
                    for rc in range(4):
                        xt = xpool.tile([P, D], f32, name=f"x{t}_{rc}", tag=f"x{t}_{rc}")
                        nc.sync.dma_start(xt[:], x_sh[t][rc * P:(rc + 1) * P, :])
                        xts.append(xt)
                    rs, nm = [], []
                    for rc in range(4):
                        r_, n_ = ln_stats_tile(spool, xts[rc], scr,
                                               Am[:, 4 * t + rc:4 * t + rc + 1],
                                               f"s0_{t}_{rc}")
                        rs.append(r_)
                        nm.append(n_)
                    xt_all.append(xts)
                    rst_all.append(rs)
                    nmu_all.append(nm)
                    if t == 2:
                        emit_weight_loads()
                # per-tensor max -> its own tiny AllReduce, so quantization
                # of q starts without waiting for k/v stats
                Am3 = xpool.tile([P, 3], f32, name="Am3", tag="Am3")
                arow = xpool.tile([1, 8], f32, name="arow", tag="arow")
                for t in range(3):
                    nc.vector.tensor_reduce(Am3[:, t:t + 1],
                                            Am[:, 4 * t:4 * t + 4],
                                            axis=AXX, op=ALU.max)
                    stage = xpool.tile([1, 8], f32, name=f"stage{t}",
                                       tag=f"stage{t}")
                    nc.vector.memset(stage[:], 0.0)
                    tree_max(spool, pst0, Am3[:, t:t + 1], 1, stage)
                    nc.sync.dma_start(ar1_in[0:1, t:t + 1], stage[0:1, 0:1])
                    nc.gpsimd.collective_compute(
                        "AllReduce", ALU.max, replica_groups=groups8,
                        ins=[ar1_in[0:1, t:t + 1].opt()],
                        outs=[ar1_out[0:1, t:t + 1].opt()])
                    nc.sync.dma_start(arow[0:1, t:t + 1],
                                      ar1_out[0:1, t:t + 1])
                for t in range(3):
                    psb = pst0.tile([P, 1], f32, name="psb", tag="psb")
                    nc.tensor.matmul(psb[:], ones1[:], arow[:, t:t + 1],
                                     start=True, stop=True)
                    nc.vector.tensor_copy(Gb[:, t:t + 1], psb[:])
                    s128 = qpool.tile([P, 1], f32, name="s128", tag="s128")
                    ig_t, _ = rsqrt_nr(qpool, Gb[:, t:t + 1], f"q{t}")
                    nc.vector.tensor_scalar(s128[:], ig_t[:], 128.0, None,
                                            ALU.mult)
                    for rc in range(4):
                        dsts = [ag_in[t * D + ic * P:t * D + (ic + 1) * P,
                                      rc * P:(rc + 1) * P] for ic in range(8)]
                        quant_tile(qpool, pst0, xt_all[t][rc], rst_all[t][rc],
                                   nmu_all[t][rc], s128, dsts, f"q{t}{rc}")
                    nc.gpsimd.collective_compute(
                        "AllGather", ALU.bypass, replica_groups=groups4,
                        ins=[ag_in[t * D:(t + 1) * D, :].opt()],
                        outs=[ag_out3[t].opt()])

                # exp scale C = gq*gk*bq*bk/(128*128*8);  iCv = 128/(gv*bv)
                bb = xpool.tile([P, 8], f32, name="bb", tag="bb")
                psb2 = pst0.tile([P, 8], f32, name="psb2", tag="psb2")
                nc.tensor.matmul(psb2[:], ones1[:], beta_sb[:], start=True,
                                 stop=True)
                nc.vector.tensor_copy(bb[:], psb2[:])
                ct1 = xpool.tile([P, 1], f32, name="ct1", tag="ct1")
                ct2 = xpool.tile([P, 1], f32, name="ct2", tag="ct2")
                _, glq = rsqrt_nr(xpool, Gb[:, 0:1], "lq")
                _, glk = rsqrt_nr(xpool, Gb[:, 1:2], "lk")
                _, glv = rsqrt_nr(xpool, Gb[:, 2:3], "lv")
                nc.vector.tensor_tensor(ct1[:], glq[:], glk[:], ALU.mult)
                nc.vector.tensor_tensor(ct2[:], bb[:, 0:1], bb[:, 1:2], ALU.mult)
                nc.vector.tensor_tensor(ct1[:], ct1[:], ct2[:], ALU.mult)
                nc.vector.tensor_scalar(Cq[:], ct1[:],
                                        1.0 / (128.0 * 128.0 * 8.0), None,
                                        ALU.mult)
                ct3 = xpool.tile([P, 1], f32, name="ct3", tag="ct3")
                nc.vector.tensor_tensor(ct3[:], glv[:], bb[:, 2:3], ALU.mult)
                nc.vector.tensor_scalar(iCv[:], ct3[:], 1.0 / 128.0, None,
                                        ALU.mult)

            # ================= stage 3+4: projections + attention ============
            def agx(t, ic, sc, c0, cn):
                r0 = D * sc + P * ic
                return ag_out3[t][r0:r0 + P, c0:c0 + cn]

            # pools spanning attention + deferred stage-5 stats (s5* stay
            # open past attper for the tail)
            with tc.tile_pool(name="attper", bufs=1) as attper:
                qpT = [attper.tile([65, S], f32r, name=f"qpT{h}", tag=f"qpT{h}") for h in range(4)]
                kpT = [attper.tile([65, S], f32r, name=f"kpT{h}", tag=f"kpT{h}") for h in range(4)]
                qpA = [attper.tile([64, S], bf16, name=f"qpA{h}", tag=f"qpA{h}") for h in range(4)]
                kpA = [attper.tile([64, S], bf16, name=f"kpA{h}", tag=f"kpA{h}") for h in range(4)]
                vps = attper.tile([P, 16 * 260], bf16, name="vps", tag="vps")

                with tc.tile_pool(name="rhsp", bufs=4) as rhsp, \
                     tc.tile_pool(name="psp3", bufs=4, space="PSUM") as psp3:
                    for h in range(4):
                        nc.sync.dma_start(kpT[h][64:65, :], ones_row)
                    for t, wsb, dstT, dstA in ((0, wq_sb, qpT, qpA),
                                               (1, wk_sb, kpT, kpA)):
                        for sc in range(4):
                            rhs = [rhsp.tile([P, SH], bf16, name=f"rh{ic % 4}", tag=f"rh{ic % 4}")
                                   for ic in range(8)]
                            for ic in range(8):
                                r8 = rhsp.tile([P, SH], mybir.dt.int8,
                                               name=f"r8{ic % 4}",
                                               tag=f"r8{ic % 4}")
                                nc.sync.dma_start(r8[:],
                                                  agx(t, ic, sc, 0, SH))
                                nc.vector.tensor_copy(rhs[ic][:], r8[:])
                            for oh in range(2):
                                ps = psp3.tile([P, SH], f32, name="ps3", tag="ps3")
                                for ic in range(8):
                                    nc.tensor.matmul(
                                        ps[:], wsb[ic][:, oh * P:(oh + 1) * P],
                                        rhs[ic][:], start=(ic == 0),
                                        stop=(ic == 7))
                                hA, hB = oh * 2, oh * 2 + 1
                                nc.scalar.activation(
                                    dstT[hA][0:64, sc * SH:(sc + 1) * SH],
                                    ps[0:64, :], CPY)
                                nc.vector.tensor_copy(
                                    dstA[hA][0:64, sc * SH:(sc + 1) * SH],
                                    ps[0:64, :])
                                stg = rhsp.tile([P, SH], f32r, name="stg", tag="stg")
                                stgb = rhsp.tile([P, SH], bf16, name="stgb", tag="stgb")
                                nc.scalar.activation(stg[64:P, :],
                                                     ps[64:P, :], CPY)
                                nc.vector.tensor_copy(stgb[64:P, :], ps[64:P, :])
                                nc.sync.dma_start(
                                    dstT[hB][0:64, sc * SH:(sc + 1) * SH],
                                    stg[64:P, :])
                                nc.sync.dma_start(
                                    dstA[hB][0:64, sc * SH:(sc + 1) * SH],
                                    stgb[64:P, :])
                    for sc4 in range(4):
                      vstr = [rhsp.tile([P, SH], bf16, name=f"vs{ic % 4}",
                                        tag=f"vs{ic % 4}") for ic in range(8)]
                      for ic in range(8):
                          v8 = rhsp.tile([P, SH], mybir.dt.int8,
                                         name=f"v8{ic % 4}", tag=f"v8{ic % 4}")
                          nc.sync.dma_start(v8[:], agx(2, ic, sc4, 0, SH))
                          nc.vector.tensor_copy(vstr[ic][:], v8[:])
                      for qc in range(4):
                        kc = sc4 * 4 + qc
                        ps = psp3.tile([P, 256], f32, name="psv", tag="psv")
                        for ic in range(8):
                            nc.tensor.matmul(ps[:],
                                             vstr[ic][:, qc * P:(qc + 1) * P],
                                             wv_sb[ic][:],
                                             start=(ic == 0), stop=(ic == 7))
                        for h in range(4):
                            nc.scalar.activation(
                                vps[:, 260 * kc + 65 * h:260 * kc + 65 * h + 64],
                                ps[:, 64 * h:64 * h + 64], CPY)
                            nc.scalar.activation(
                                vps[:, 260 * kc + 65 * h + 64:
                                    260 * kc + 65 * h + 65],
                                ones128[:], CPY)

                # ---- attention: singles-granularity software pipeline ----
                # per kc step of block i the PE runs [A(i+2,kc), B(i,kc),
                # AV(i,kc-4)]; exp fires per B single; pass A runs two blocks
                # ahead so the -M row lands a full block before B reads it;
                # the epilogue of block i-1 (ACT copy, PE transposes, DVE
                # division) is stitched into block i's steps 0-7 so the PE
                # never drains at a block boundary (keeps the 2.4GHz p-state).
                # PSUM: psA 2x[128,1024] + psB 2x[128,512] + psX + psT = 8.
                with tc.tile_pool(name="mpool", bufs=3) as mpool, \
                     tc.tile_pool(name="ptp", bufs=8) as ptp, \
                     tc.tile_pool(name="trp", bufs=2) as trp, \
                     tc.tile_pool(name="xtsb", bufs=2) as xtsb, \
                     tc.tile_pool(name="psA", bufs=2, space="PSUM") as psA, \
                     tc.tile_pool(name="psB", bufs=2, space="PSUM") as psB, \
                     tc.tile_pool(name="psX", bufs=1, space="PSUM") as psX, \
                     tc.tile_pool(name="psT", bufs=1, space="PSUM") as psT:

                    blocks = [(h, qb) for qb in range(4) for h in range(4)]

                    mp_tiles = {}
                    xps_tiles = {}
                    pa_cur = [None]
                    pT_lists = {}

                    def new_xps(blk):
                        xps_tiles[blk] = psX.tile([65, SH], f32,
                                                  name="xps", tag="xps")

                    def emit_A_step(blk, s):
                        """one bf16 QK single [128 q x 512 k] into a bf16
                        PSUM pair-tile half; the row-max estimate is built
                        with a bf16 tensor-tensor max tree (2x DVE mode; a
                        plain PSUM f32 reduce would pace the whole pipeline
                        below the 2.4GHz PE p-state). bf16 rounding of the
                        logits costs <~25 on the estimate; pass B's exact
                        logits-minus-M keeps exp args bounded either way."""
                        h, qb = blk
                        q0 = qb * SH
                        qc, kb = s // 4, s % 4
                        if kb == 0:
                            mp_tiles[blk + (qc,)] = mpool.tile(
                                [P, 2], f32, name="Mp", tag="Mp")
                        if s % 2 == 0:
                            pa_cur[0] = psA.tile([P, 2 * SH], f32, name="pa",
                                                 tag="pa")
                        pa = pa_cur[0]
                        half = s % 2
                        nc.tensor.matmul(
                            pa[:, half * SH:(half + 1) * SH],
                            qpA[h][:, q0 + qc * P:q0 + (qc + 1) * P],
                            kpA[h][:, kb * SH:(kb + 1) * SH],
                            start=True, stop=True)
                        Mp = mp_tiles[blk + (qc,)]
                        if half == 1:
                            nc.vector.tensor_reduce(
                                Mp[:, kb // 2:kb // 2 + 1], pa[:], axis=AXX,
                                op=ALU.max)
                        if kb == 3:
                            ngm = mpool.tile([P, 1], f32r, name="ngm", tag="ngm")
                            nc.vector.tensor_reduce(ngm[:], Mp[:], axis=AXX,
                                                    op=ALU.max, negate=True)
                            nc.sync.dma_start(
                                qpT[h][64:65, q0 + qc * P:q0 + (qc + 1) * P],
                                ngm[:])
                            mp_tiles.pop(blk + (qc,))

                    def emit_B(blk, kc):
                        """one f32r QK single [128 k x 512 q] + its exp."""
                        h, qb = blk
                        q0 = qb * SH
                        pb = psB.tile([P, SH], f32, name="pb", tag="pb")
                        nc.tensor.matmul(
                            pb[:],
                            kpT[h][:, kc * P:(kc + 1) * P],
                            qpT[h][:, q0:q0 + SH],
                            start=True, stop=True)
                        pT = ptp.tile([P, SH], bf16, name="pT", tag="pT")
                        nc.scalar.activation(pT[:], pb[:], EXP, scale=Cq[:])
                        pT_lists[blk].append(pT)

                    def emit_AV(blk, kc):
                        h, qb = blk
                        nc.tensor.matmul(
                            xps_tiles[blk],
                            vps[:, 260 * kc + 65 * h:260 * kc + 65 * (h + 1)],
                            pT_lists[blk][kc],
                            start=(kc == 0), stop=(kc == 15))

                    def emit_epi_copy(blk):
                        """ACT copy drains xps -> SBUF, freeing the psX bank."""
                        xps = xps_tiles.pop(blk)
                        xt_s = xtsb.tile([65, SH], f32, name="xt_s", tag="xt_s")
                        nc.scalar.activation(xt_s[:], xps[:], CPY)
                        pT_lists.pop(blk)
                        return xt_s

                    def emit_epi_div(blk, xt_s, qc):
                        """transpose one 128-query chunk + divide by denom."""
                        h, qb = blk
                        ptx = psT.tile([P, 65], f32, name="ptx", tag="ptx")
                        nc.tensor.transpose(
                            ptx[:], xt_s[0:65, qc * P:(qc + 1) * P],
                            idf[0:65, 0:65])
                        rv = mpool.tile([P, 1], f32, name="rv", tag="rv")
                        rv0 = mpool.tile([P, 1], f32, name="rv0", tag="rv0")
                        nc.vector.reciprocal(rv0[:], ptx[:, 64:65])
                        nc.vector.tensor_tensor(rv[:], rv0[:], iCv[:],
                                                ALU.mult)
                        nc.vector.tensor_scalar(
                            xsb[qb * 4 + qc][:, 64 * h:64 * (h + 1)],
                            ptx[:, 0:64], rv[:], None, ALU.mult)

                    def emit_xag(qb):
                        for qc in range(4):
                            # scalar-triggered: keeps these off the sync
                            # stream, where pending -M DMAs would HOL-block
                            # them for more than a block
                            nc.scalar.dma_start(
                                xag_in[qb * SH + qc * P:
                                       qb * SH + (qc + 1) * P, :],
                                xsb[qb * 4 + qc][:])
                        nc.gpsimd.collective_compute(
                            "AllGather", ALU.bypass, replica_groups=groups4,
                            ins=[xag_in[qb * SH:(qb + 1) * SH, :].opt()],
                            outs=[xag_out4[qb].opt()])

                    # prologue: pass A for blocks 0 and 1
                    for s in range(16):
                        emit_A_step(blocks[0], s)
                    for s in range(16):
                        emit_A_step(blocks[1], s)

                    for i, blk in enumerate(blocks):
                        h, qb = blk
                        prv = blocks[i - 1] if i > 0 else None
                        nxt2 = blocks[i + 2] if i + 2 < len(blocks) else None
                        new_xps(blk)
                        pT_lists[blk] = []
                        xt_prev = [None]
                        for kc in range(16):
                            if prv is not None and kc < 4:
                                emit_AV(prv, 12 + kc)
                                if kc == 3:
                                    xt_prev[0] = emit_epi_copy(prv)
                            emit_B(blk, kc)
                            if nxt2 is not None:
                                emit_A_step(nxt2, kc)
                            if 4 <= kc < 8 and prv is not None:
                                emit_epi_div(prv, xt_prev[0], kc - 4)
                                if kc == 7 and prv[0] == 3:
                                    # previous qb group complete: ship its x
                                    emit_xag(prv[1])
                            if kc >= 4:
                                emit_AV(blk, kc - 4)
                        # spread deferred x-stats one tile per block so the
                        # DVE never pushes the block pace above the PE's;
                        # late blocks take a second tile to shorten the tail
                        if i >= 6:
                            emit_x_stat_tile(i - 6, mpool)
                        if i >= 14:
                            emit_x_stat_tile(i - 4, mpool)

                    # flush: last block's AV tail + epilogue + its qb AG
                    lst = blocks[-1]
                    for kc in range(12, 16):
                        emit_AV(lst, kc)
                    xt_l = emit_epi_copy(lst)
                    for qc in range(4):
                        emit_epi_div(lst, xt_l, qc)
                    emit_xag(lst[1])

                if DEBUG:
                    with tc.tile_pool(name="dbgp", bufs=1) as dbgp:
                        nc.sync.dma_start(dbg_qpa, qpA[0][:, 0:256])
                        nc.sync.dma_start(dbg_kpa, kpA[0][:, 0:256])
                        nc.sync.dma_start(dbg_qpt, qpT[0][:, 0:256])
                        nc.sync.dma_start(dbg_kpt, kpT[0][:, 0:256])
                        nc.sync.dma_start(dbg_vps, vps[:, 0:260])
                        nc.sync.dma_start(dbg_xsb, xsb[0][:])

            # ====== stage 5 tail: last stats, AR2, quant, projection =====
            with tc.tile_pool(name="fpool", bufs=1) as fpool, \
                 tc.tile_pool(name="f2pool", bufs=3) as f2pool, \
                 tc.tile_pool(name="psf", bufs=1, space="PSUM") as psf:
                for rc in range(12, 16):
                    emit_x_stat_tile(rc, f2pool)
                xq0T = fpool.tile([P, 16 * D], bf16, name="xq0T", tag="xq0T")
                Am21 = fpool.tile([P, 1], f32, name="Am21", tag="Am21")
                nc.vector.tensor_reduce(Am21[:], Am2[:], axis=AXX, op=ALU.max)
                stage2 = fpool.tile([1, 8], f32, name="stage2", tag="stage2")
                nc.vector.memset(stage2[:], 0.0)
                tree_max(f2pool, psf, Am21, 1, stage2)
                nc.sync.dma_start(ar2_in[:], stage2[:])
                nc.gpsimd.collective_compute(
                    "AllReduce", ALU.max, replica_groups=groups8,
                    ins=[ar2_in.opt()], outs=[ar2_out.opt()])
                arow2 = fpool.tile([1, 8], f32, name="arow2", tag="arow2")
                nc.sync.dma_start(arow2[:], ar2_out[:])
                psb3 = psf.tile([P, 8], f32, name="psb3", tag="psb3")
                nc.tensor.matmul(psb3[:], ones1[:], arow2[:], start=True,
                                 stop=True)
                G2 = fpool.tile([P, 8], f32, name="G2", tag="G2")
                nc.vector.tensor_copy(G2[:], psb3[:])
                s128b = fpool.tile([P, 1], f32, name="s128b", tag="s128b")
                ig0, gl0 = rsqrt_nr(fpool, G2[:, 0:1], "f0")
                nc.vector.tensor_scalar(s128b[:], ig0[:], 128.0, None,
                                        ALU.mult)
                bb2 = fpool.tile([P, 8], f32, name="bb2", tag="bb2")
                psb4 = psf.tile([P, 8], f32, name="psb4", tag="psb3")
                nc.tensor.matmul(psb4[:], ones1[:], beta_sb[:], start=True,
                                 stop=True)
                nc.vector.tensor_copy(bb2[:], psb4[:])
                C0 = fpool.tile([P, 1], f32, name="C0", tag="C0")
                nc.vector.tensor_tensor(C0[:], gl0[:], bb2[:, 3:4], ALU.mult)
                nc.vector.tensor_scalar(C0[:], C0[:], 1.0 / 128.0, None,
                                        ALU.mult)
                # quant + flipped output projection, interleaved per 4-row
                # group so the PE matmuls overlap the next group's quant
                xq4 = xq0T[:].rearrange("p (rc ic c) -> p rc ic c",
                                        rc=16, ic=8, c=P)
                for r4 in range(4):
                    for rc in range(4 * r4, 4 * r4 + 4):
                        xf = f2pool.tile([P, D], bf16, name="xf2", tag="xf2")
                        qb = rc // 4
                        for j in range(4):
                            nc.sync.dma_start(
                                xf[:, 256 * j:256 * (j + 1)],
                                xag_out4[qb][SH * j + (rc % 4) * P:
                                             SH * j + (rc % 4 + 1) * P, :])
                        quant_tile(f2pool, psf, xf, rst2[rc], nmu2[rc],
                                   s128b, xq0T[:, rc * D:(rc + 1) * D],
                                   f"f{rc}")
                    for oh in range(2):
                        ps = psf.tile([P, SH], f32, name="pso", tag="pso",
                                      bufs=2)
                        for ic in range(8):
                            nc.tensor.matmul(
                                ps[:],
                                w0_sb[ic][:, oh * P:(oh + 1) * P],
                                xq4[:, 4 * r4:4 * (r4 + 1), ic, :],
                                start=(ic == 0), stop=(ic == 7))
                        yt = f2pool.tile([P, SH], f32, name="yt", tag="yt")
                        nc.scalar.activation(yt[:], ps[:], CPY, scale=C0[:])
                        nc.sync.dma_start(
                            y[oh * P:(oh + 1) * P, r4 * SH:(r4 + 1) * SH],
                            yt[:])

    nc.compile()
    return nc


def _prep_host(inputs):
    import ml_dtypes
    bf = ml_dtypes.bfloat16
    ws = [inputs["wq_w"], inputs["wk_w"], inputs["wv_w"], inputs["w0_w"]]
    signs = []
    betas = np.zeros((1, 8), np.float32)
    for i, w in enumerate(ws):
        w64 = np.asarray(w, np.float64)
        signs.append(np.sign(w64 - w64.mean()).astype(np.float32))
        betas[0, i] = np.abs(w64).mean()
    id_bf = np.eye(128, dtype=bf)
    id_f = np.eye(128, dtype=np.float32)
    ones_row = np.ones((1, S), np.float32)
    qf = np.asarray(inputs["q"], np.float32).reshape(2 * S, D)
    kf = np.asarray(inputs["k"], np.float32).reshape(2 * S, D)
    vf = np.asarray(inputs["v"], np.float32).reshape(2 * S, D)
    in_maps = []
    for c in range(8):
        b, g = c // 4, c % 4
        r0 = b * S + g * SH
        m = {
            "x0": np.ascontiguousarray(qf[r0:r0 + SH]),
            "x1": np.ascontiguousarray(kf[r0:r0 + SH]),
            "x2": np.ascontiguousarray(vf[r0:r0 + SH]),
            "wo": np.ascontiguousarray(
                signs[3].T[:, 256 * g:256 * (g + 1)]).astype(bf),
            "idbf": id_bf, "idf": id_f,
            "onesrow": ones_row, "betas": betas,
        }
        for t in range(3):
            m[f"w{t}"] = np.ascontiguousarray(
                signs[t].T[:, 256 * g:256 * (g + 1)]).astype(bf)
        in_maps.append(m)
    return in_maps


def _run(inputs, trace=False):
    global _COMPILED
    from concourse import bass_utils
    if _COMPILED is None:
        _COMPILED = _build()
    nc = _COMPILED
    in_maps = _prep_host(inputs)
    res = bass_utils.run_bass_kernel_spmd(nc, in_maps, core_ids=list(range(8)),
                                          trace=trace)
    out = np.zeros((B, S, D), np.float32)
    for c in range(8):
        b, g = c // 4, c % 4
        out[b, :, 256 * g:256 * (g + 1)] = res.results[c]["y"].T
    return out, res


def kernel(**inputs):
    mask = np.asarray(inputs["mask"])
    if not (mask == 1).all():
        return _numpy_fallback(**inputs)
    out, _ = _run(inputs, trace=False)
    return out


def _numpy_fallback(q, k, v, mask, wq_w, wk_w, wv_w, w0_w):
    f = np.float32

    def ln(x):
        mu = x.mean(-1, keepdims=True, dtype=f)
        var = np.mean((x - mu) ** 2, -1, keepdims=True, dtype=f)
        return ((x - mu) / np.sqrt(var + f(LN_EPS))).astype(f)

    def bitlin(x, w):
        xn = ln(np.asarray(x, f))
        mx = np.abs(xn).max()
        xq = np.round(xn * (f(128.0) / mx)).astype(f)
        xq = (np.mod(xq + 128.0, 256.0) - 128.0).astype(f)
        wq = np.sign(w - w.mean(dtype=f)).astype(f)
        beta = np.abs(w).mean(dtype=f)
        return ((xq @ wq.T) * f(mx / 128 * beta)).astype(f)

    qp = bitlin(q, wq_w).reshape(B, S, H, DK).transpose(0, 2, 1, 3)
    kp = bitlin(k, wk_w).reshape(B, S, H, DK).transpose(0, 2, 1, 3)
    vp = bitlin(v, wv_w).reshape(B, S, H, DK).transpose(0, 2, 1, 3)
    out = np.zeros((B, H, S, DK), f)
    mask = np.asarray(mask)
    for b in range(B):
        for h in range(H):
            att = (qp[b, h] @ kp[b, h].T) / f(np.sqrt(DK))
            att = np.where(mask[b] == 0, f(-1e9), att).astype(f)
            att = att - att.max(-1, keepdims=True)
            e = np.exp(att)
            p = e / e.sum(-1, keepdims=True)
            out[b, h] = p @ vp[b, h]
    x = out.transpose(0, 2, 1, 3).reshape(B, S, H * DK)
    return bitlin(x, w0_w)



# revision 35
# speedup vs baseline: 1.0990x; 1.0680x over previous
"""BitLinear multi-head attention on 8 trn2 NeuronCores.

Sharding: core c handles batch b=c//4 and head group g=c%4 (heads 4g..4g+3).
Stages:
  0. dummy collective at t=0 absorbs the first-collective barrier under the
     input loads + LN stats of the core's 512-row shard of q/k/v
  1. AllReduce(max) of the 3 global absmax scalars
  2. per tensor: quantize to int8-valued bf16, PE-transpose, AllGather
     (three pipelined collectives so projections overlap the later gathers)
  3. projections qpT/kpT (transposed, integer-exact bf16 matmuls) and vp
     (+ bf16 shadow copies qpA/kpA for the max-estimate pass)
  4. attention blocks (h inner, qb outer), software-pipelined; per block the
     PE runs dtype-uniform batches to avoid mode-switch stalls:
       [16 f32r QK matmuls (pass B, paired into [128,1024] PSUM) + 8 exps]
       [16 bf16 att@V matmuls] [16 bf16 pass-A matmuls of the NEXT block]
     pass A estimates the per-query max in bf16 (exp arg stays <= ~+7, safe);
     DVE negated-max reduces write -M via DMA into qpT's augmented row; pass
     B's ones-row/-M-row trick yields exact logits-minus-max; the ones column
     of vps gives the softmax denominator.  After each qb group finishes, its
     x slice is AllGathered (hidden under the next group's compute) and its
     LN stats are emitted one group deferred.
  5. tail: AR2 (global absmax), streamed re-quantization of x, output
     projection (feature-sharded); host assembles the final tensor.
"""
import numpy as np

B, S, D, H, DK = 2, 2048, 1024, 16, 64
SH = 512            # rows per core shard (stage 0)
P = 128
MAGIC = 12582912.0  # 1.5 * 2**23, forces RNE-round-to-int for |x| < 2**22
LN_EPS = 1e-5

_COMPILED = None
DEBUG = False


def _build():
    import concourse.tile as tile
    from concourse import bacc, mybir

    f32 = mybir.dt.float32
    f32r = mybir.dt.float32r
    bf16 = mybir.dt.bfloat16
    EXP = mybir.ActivationFunctionType.Exp
    CPY = mybir.ActivationFunctionType.Copy
    IDN = mybir.ActivationFunctionType.Identity
    SQ = mybir.ActivationFunctionType.Square
    SQRT = mybir.ActivationFunctionType.Sqrt
    ALU = mybir.AluOpType
    AXX = mybir.AxisListType.X

    nc = bacc.Bacc("TRN2", target_bir_lowering=False, debug=False,
                   enable_asserts=False, num_devices=8)

    x_sh = [nc.dram_tensor(f"x{t}", [SH, D], f32, kind="ExternalInput").ap()
            for t in range(3)]
    w_t = [nc.dram_tensor(f"w{t}", [D, 256], bf16, kind="ExternalInput").ap()
           for t in range(3)]                       # sign(w)^T o-slices, bf16
    w0_t = nc.dram_tensor("wo", [D, 256], bf16, kind="ExternalInput").ap()
    id_bf = nc.dram_tensor("idbf", [P, P], bf16, kind="ExternalInput").ap()
    id_f = nc.dram_tensor("idf", [P, P], f32, kind="ExternalInput").ap()
    ones_row = nc.dram_tensor("onesrow", [1, S], f32r, kind="ExternalInput").ap()
    betas = nc.dram_tensor("betas", [1, 8], f32, kind="ExternalInput").ap()
    y = nc.dram_tensor("y", [256, S], f32, kind="ExternalOutput").ap()
    if DEBUG:
        dbg_qpa = nc.dram_tensor("dbg_qpa", [64, 256], bf16, kind="ExternalOutput").ap()
        dbg_kpa = nc.dram_tensor("dbg_kpa", [64, 256], bf16, kind="ExternalOutput").ap()
        dbg_qpt = nc.dram_tensor("dbg_qpt", [65, 256], f32r, kind="ExternalOutput").ap()
        dbg_kpt = nc.dram_tensor("dbg_kpt", [65, 256], f32r, kind="ExternalOutput").ap()
        dbg_vps = nc.dram_tensor("dbg_vps", [P, 260], bf16, kind="ExternalOutput").ap()
        dbg_xsb = nc.dram_tensor("dbg_xsb", [P, 256], f32, kind="ExternalOutput").ap()

    groups8 = [list(range(8))]
    groups4 = [[0, 1, 2, 3], [4, 5, 6, 7]]

    with tile.TileContext(nc) as tc:
        with tc.tile_pool(name="dram", bufs=1, space="DRAM") as dram, \
             tc.tile_pool(name="persist", bufs=1) as per:

            # ---- always-live SBUF ----
            xsb = [per.tile([P, 256], bf16, name=f"xsb{i}", tag=f"xsb{i}") for i in range(16)]
            Gb = per.tile([P, 8], f32, name="Gb", tag="Gb")
            Cq = per.tile([P, 1], f32, name="Cq", tag="Cq")
            iCv = per.tile([P, 1], f32, name="iCv", tag="iCv")
            idb = per.tile([P, P], bf16, name="idb", tag="idb")
            idf = per.tile([P, P], f32, name="idf", tag="idf")
            beta_sb = per.tile([1, 8], f32, name="beta_sb", tag="beta_sb")
            ones1 = per.tile([1, P], f32, name="ones1", tag="ones1")
            ones128 = per.tile([P, 1], f32, name="ones128", tag="ones128")
            wq_sb = [per.tile([P, 256], bf16, name=f"wq{ic}", tag=f"wq{ic}")
                     for ic in range(8)]
            wk_sb = [per.tile([P, 256], bf16, name=f"wk{ic}", tag=f"wk{ic}")
                     for ic in range(8)]
            wv_sb = [per.tile([P, 256], bf16, name=f"wv{ic}", tag=f"wv{ic}")
                     for ic in range(8)]
            w0_sb = [per.tile([P, 256], bf16, name=f"w0{ic}", tag=f"w0{ic}")
                     for ic in range(8)]
            nc.sync.dma_start(idb[:], id_bf)
            nc.sync.dma_start(idf[:], id_f)
            nc.sync.dma_start(beta_sb[:], betas)
            nc.vector.memset(ones1[:], 1.0)
            nc.vector.memset(ones128[:], 1.0)

            def emit_weight_loads():
                # emitted after the x-shard loads: the sync queue drains in
                # order, and stats must not wait behind 8MB of weights
                for ic in range(8):
                    nc.sync.dma_start(wq_sb[ic][:],
                                      w_t[0][ic * P:(ic + 1) * P, :])
                    nc.sync.dma_start(wk_sb[ic][:],
                                      w_t[1][ic * P:(ic + 1) * P, :])
                    nc.sync.dma_start(wv_sb[ic][:],
                                      w_t[2][ic * P:(ic + 1) * P, :])
                    nc.sync.dma_start(w0_sb[ic][:],
                                      w0_t[ic * P:(ic + 1) * P, :])

            # DRAM bounce buffers
            i8 = mybir.dt.int8
            ag_in = dram.tile([3 * D, SH], i8, name="ag_in", tag="ag_in")
            ag_out3 = [dram.tile([4 * D, SH], i8, name=f"ago{t}", tag=f"ago{t}")
                       for t in range(3)]
            ar1_in = dram.tile([1, 8], f32, name="ar1_in", tag="ar1_in")
            ar1_out = dram.tile([1, 8], f32, name="ar1_out", tag="ar1_out")
            ar2_in = dram.tile([1, 8], f32, name="ar2_in", tag="ar2_in")
            ar2_out = dram.tile([1, 8], f32, name="ar2_out", tag="ar2_out")
            dmy_in = dram.tile([1, 8], f32, name="dmy_in", tag="dmy_in")
            dmy_out = dram.tile([1, 8], f32, name="dmy_out", tag="dmy_out")
            xag_in = dram.tile([S, 256], bf16, name="xag_in", tag="xag_in")
            xag_out4 = [dram.tile([4 * SH, 256], bf16, name=f"xao{j}",
                                  tag=f"xao{j}") for j in range(4)]


            def ln_stats_tile(pool, xtile, scr, am_out, sfx):
                """LN stats for one [P, D] tile. Returns (rstd, negmu) tiles."""
                sm = pool.tile([P, 1], f32, name="sm", tag="sm")
                nmu = pool.tile([P, 1], f32, name=f"nmu{sfx}", tag=f"nmu{sfx}")
                ssq = pool.tile([P, 1], f32, name="ssq", tag="ssq")
                rst = pool.tile([P, 1], f32, name=f"rst{sfx}", tag=f"rst{sfx}")
                mxs = pool.tile([P, 1], f32, name="mxs", tag="mxs")
                r2 = pool.tile([P, 1], f32, name="r2", tag="r2")
                nc.vector.tensor_reduce(sm[:], xtile[:], axis=AXX, op=ALU.add)
                nc.scalar.activation(nmu[:], sm[:], CPY, scale=-1.0 / D)
                nc.scalar.activation(scr[:], xtile[:], SQ, bias=nmu[:],
                                     accum_out=ssq[:])
                var_ = pool.tile([P, 1], f32, name="var_", tag="var_")
                rvar = pool.tile([P, 1], f32, name="rvar", tag="rvar")
                y0 = pool.tile([P, 1], f32, name="y0", tag="y0")
                nr = pool.tile([P, 1], f32, name="nr", tag="nr")
                nc.scalar.activation(var_[:], ssq[:], CPY, scale=1.0 / D,
                                     bias=LN_EPS)
                nc.vector.reciprocal(rvar[:], var_[:])
                nc.scalar.activation(y0[:], rvar[:], SQRT)
                # two Newton steps: rstd = y0*(1.5 - 0.5*var*y0^2)
                nc.vector.tensor_tensor(nr[:], y0[:], y0[:], ALU.mult)
                nc.vector.tensor_tensor(nr[:], nr[:], var_[:], ALU.mult)
                nc.vector.tensor_scalar(nr[:], nr[:], -0.5, 1.5, ALU.mult,
                                        ALU.add)
                nc.vector.tensor_tensor(y0[:], y0[:], nr[:], ALU.mult)
                nc.vector.tensor_tensor(nr[:], y0[:], y0[:], ALU.mult)
                nc.vector.tensor_tensor(nr[:], nr[:], var_[:], ALU.mult)
                nc.vector.tensor_scalar(nr[:], nr[:], -0.5, 1.5, ALU.mult,
                                        ALU.add)
                nc.vector.tensor_tensor(rst[:], y0[:], nr[:], ALU.mult)
                nc.vector.tensor_reduce(mxs[:], scr[:], axis=AXX, op=ALU.max)
                nc.vector.tensor_tensor(r2[:], rst[:], rst[:], ALU.mult)
                # squared row amax: max((x-mu)^2) * rstd^2  (no sqrt here)
                nc.vector.tensor_tensor(am_out, mxs[:], r2[:], ALU.mult)
                return rst, nmu

            def tree_max(pool, pspool, src, ncols, out_row):
                """Partition-axis max over src[:, :ncols] -> out_row[0:1, :ncols]."""
                ptr = pspool.tile([P, P], f32, name="trps", tag="trps")
                nc.tensor.transpose(ptr[0:ncols, :], src[:, :ncols], idf[:])
                a = pool.tile([P, P], f32, name="trA", tag="trA")
                nc.scalar.activation(a[0:ncols, :], ptr[0:ncols, :], CPY)
                m = pool.tile([P, 1], f32, name="trM", tag="trM")
                nc.vector.tensor_reduce(m[0:ncols, :], a[0:ncols, :], axis=AXX,
                                        op=ALU.max)
                nc.sync.dma_start(out_row[0:1, 0:ncols], m[0:ncols, :])

            def rsqrt_nr(pool, gsq_ap, sfx):
                """accurate rsqrt of a [P,1] squared-max -> (1/g, g) tiles."""
                rv_ = pool.tile([P, 1], f32, name=f"rv{sfx}", tag=f"rv{sfx}")
                yy = pool.tile([P, 1], f32, name=f"yy{sfx}", tag=f"yy{sfx}")
                tn = pool.tile([P, 1], f32, name=f"tn{sfx}", tag=f"tn{sfx}")
                ig = pool.tile([P, 1], f32, name=f"ig{sfx}", tag=f"ig{sfx}")
                gl = pool.tile([P, 1], f32, name=f"gl{sfx}", tag=f"gl{sfx}")
                nc.vector.reciprocal(rv_[:], gsq_ap)
                nc.scalar.activation(yy[:], rv_[:], SQRT)
                nc.vector.tensor_tensor(tn[:], yy[:], yy[:], ALU.mult)
                nc.vector.tensor_tensor(tn[:], tn[:], gsq_ap, ALU.mult)
                nc.vector.tensor_scalar(tn[:], tn[:], -0.5, 1.5, ALU.mult,
                                        ALU.add)
                nc.vector.tensor_tensor(ig[:], yy[:], tn[:], ALU.mult)
                nc.vector.tensor_tensor(gl[:], gsq_ap, ig[:], ALU.mult)
                return ig, gl

            def quant_tile(pool, pspool, xtile, rst, nmu, s128, dst_tiles, sfx):
                """Quantize one [P, D] tile -> 8 transposed [P, P] bf16 writes."""
                sc_ = pool.tile([P, 1], f32, name="sc_", tag="sc_")
                bp = pool.tile([P, 1], f32, name="bp", tag="bp")
                nc.vector.tensor_tensor(sc_[:], rst[:], s128[:], ALU.mult)
                nc.vector.tensor_tensor(bp[:], nmu[:], sc_[:], ALU.mult)
                # NOTE: MAGIC must NOT fold into the ACT bias — ACT's fused
                # multiply-add rounds once, flipping ints vs the reference's
                # two-step f32 rounding (verified on HW).
                t1a = pool.tile([P, D], f32, name="t1a", tag="t1a")
                t2 = pool.tile([P, D], f32, name="t2", tag="t2")
                msk = pool.tile([P, D], f32, name="msk", tag="msk")
                xqb = pool.tile([P, D], bf16, name="xqb", tag="xqb")
                nc.scalar.activation(t1a[:], xtile[:], IDN, scale=sc_[:], bias=bp[:])
                nc.scalar.activation(t2[:], t1a[:], CPY, bias=MAGIC)
                nc.vector.tensor_scalar(msk[:], t2[:], MAGIC + 127.5, 256.0,
                                        ALU.is_ge, ALU.mult)
                nc.vector.scalar_tensor_tensor(xqb[:], t2[:], -MAGIC, msk[:],
                                               ALU.add, ALU.subtract)
                pstr = pspool.tile([P, D], bf16, name="pstr", tag="pstr",
                                   bufs=2)
                for ic in range(8):
                    nc.tensor.transpose(pstr[:, ic * P:(ic + 1) * P],
                                        xqb[:, ic * P:(ic + 1) * P],
                                        idb[:])
                if isinstance(dst_tiles, list):
                    # DRAM chunk targets: one wide PSUM->SBUF cast to int8
                    # (halves the AllGather payload), then DMAs
                    stg = pool.tile([P, D], mybir.dt.int8, name="qstg",
                                    tag="qstg")
                    nc.vector.tensor_copy(stg[:], pstr[:])
                    for ic in range(8):
                        nc.sync.dma_start(dst_tiles[ic],
                                          stg[:, ic * P:(ic + 1) * P])
                else:
                    # one wide SBUF target: single DVE copy
                    nc.vector.tensor_copy(dst_tiles, pstr[:])

            Am2 = per.tile([P, 16], f32, name="Am2", tag="Am2")
            scr2 = per.tile([P, D], f32, name="scr2", tag="scr2")
            rst2 = [per.tile([P, 1], f32, name=f"rk{rc}", tag=f"rk{rc}")
                    for rc in range(16)]
            nmu2 = [per.tile([P, 1], f32, name=f"nk{rc}", tag=f"nk{rc}")
                    for rc in range(16)]

            def emit_x_stat_tile(rc, pool):
                """deferred stage-5 LN stats for one 128-row chunk of x."""
                qb = rc // 4
                xf = pool.tile([P, D], bf16, name="xf", tag="xf")
                for j in range(4):
                    # gpsimd SWDGE: a wait on the AG here must not HOL-block
                    # the sync queue that carries the -M row DMAs
                    nc.gpsimd.dma_start(
                        xf[:, 256 * j:256 * (j + 1)],
                        xag_out4[qb][SH * j + (rc % 4) * P:
                                     SH * j + (rc % 4 + 1) * P, :])
                r_, n_ = ln_stats_tile(pool, xf, scr2,
                                       Am2[:, rc:rc + 1], "s5")
                nc.vector.tensor_copy(rst2[rc][:], r_[:])
                nc.vector.tensor_copy(nmu2[rc][:], n_[:])

            # ================= stage 0-2: stats, AR, quantize, AGs ===========
            with tc.tile_pool(name="xpool", bufs=1) as xpool, \
                 tc.tile_pool(name="spool", bufs=4) as spool, \
                 tc.tile_pool(name="qpool", bufs=3) as qpool, \
                 tc.tile_pool(name="pst0", bufs=2, space="PSUM") as pst0:
                scr = spool.tile([P, D], f32, name="scr", tag="scr")
                Am = xpool.tile([P, 16], f32, name="Am", tag="Am")
                xt_all, rst_all, nmu_all = [], [], []
                for t in range(3):
                    xts = []
                    for rc in range(4):
                        xt = xpool.tile([P, D], f32, name=f"x{t}_{rc}", tag=f"x{t}_{rc}")
                        nc.sync.dma_start(xt[:], x_sh[t][rc * P:(rc + 1) * P, :])
                        xts.append(xt)
                    rs, nm = [], []
                    for rc in range(4):
                        r_, n_ = ln_stats_tile(spool, xts[rc], scr,
                                               Am[:, 4 * t + rc:4 * t + rc + 1],
                                               f"s0_{t}_{rc}")
                        rs.append(r_)
                        nm.append(n_)
                    xt_all.append(xts)
                    rst_all.append(rs)
                    nmu_all.append(nm)
                    if t == 2:
                        emit_weight_loads()
                # per-tensor max -> its own tiny AllReduce, so quantization
                # of q starts without waiting for k/v stats
                Am3 = xpool.tile([P, 3], f32, name="Am3", tag="Am3")
                arow = xpool.tile([1, 8], f32, name="arow", tag="arow")
                for t in range(3):
                    nc.vector.tensor_reduce(Am3[:, t:t + 1],
                                            Am[:, 4 * t:4 * t + 4],
                                            axis=AXX, op=ALU.max)
                    stage = xpool.tile([1, 8], f32, name=f"stage{t}",
                                       tag=f"stage{t}")
                    nc.vector.memset(stage[:], 0.0)
                    tree_max(spool, pst0, Am3[:, t:t + 1], 1, stage)
                    nc.sync.dma_start(ar1_in[0:1, t:t + 1], stage[0:1, 0:1])
                    nc.gpsimd.collective_compute(
                        "AllReduce", ALU.max, replica_groups=groups8,
                        ins=[ar1_in[0:1, t:t + 1].opt()],
                        outs=[ar1_out[0:1, t:t + 1].opt()])
                    nc.sync.dma_start(arow[0:1, t:t + 1],
                                      ar1_out[0:1, t:t + 1])
                for t in range(3):
                    psb = pst0.tile([P, 1], f32, name="psb", tag="psb")
                    nc.tensor.matmul(psb[:], ones1[:], arow[:, t:t + 1],
                                     start=True, stop=True)
                    nc.vector.tensor_copy(Gb[:, t:t + 1], psb[:])
                    s128 = qpool.tile([P, 1], f32, name="s128", tag="s128")
                    ig_t, _ = rsqrt_nr(qpool, Gb[:, t:t + 1], f"q{t}")
                    nc.vector.tensor_scalar(s128[:], ig_t[:], 128.0, None,
                                            ALU.mult)
                    for rc in range(4):
                        dsts = [ag_in[t * D + ic * P:t * D + (ic + 1) * P,
                                      rc * P:(rc + 1) * P] for ic in range(8)]
                        quant_tile(qpool, pst0, xt_all[t][rc], rst_all[t][rc],
                                   nmu_all[t][rc], s128, dsts, f"q{t}{rc}")
                    nc.gpsimd.collective_compute(
                        "AllGather", ALU.bypass, replica_groups=groups4,
                        ins=[ag_in[t * D:(t + 1) * D, :].opt()],
                        outs=[ag_out3[t].opt()])

                # exp scale C = gq*gk*bq*bk/(128*128*8);  iCv = 128/(gv*bv)
                bb = xpool.tile([P, 8], f32, name="bb", tag="bb")
                psb2 = pst0.tile([P, 8], f32, name="psb2", tag="psb2")
                nc.tensor.matmul(psb2[:], ones1[:], beta_sb[:], start=True,
                                 stop=True)
                nc.vector.tensor_copy(bb[:], psb2[:])
                ct1 = xpool.tile([P, 1], f32, name="ct1", tag="ct1")
                ct2 = xpool.tile([P, 1], f32, name="ct2", tag="ct2")
                _, glq = rsqrt_nr(xpool, Gb[:, 0:1], "lq")
                _, glk = rsqrt_nr(xpool, Gb[:, 1:2], "lk")
                _, glv = rsqrt_nr(xpool, Gb[:, 2:3], "lv")
                nc.vector.tensor_tensor(ct1[:], glq[:], glk[:], ALU.mult)
                nc.vector.tensor_tensor(ct2[:], bb[:, 0:1], bb[:, 1:2], ALU.mult)
                nc.vector.tensor_tensor(ct1[:], ct1[:], ct2[:], ALU.mult)
                nc.vector.tensor_scalar(Cq[:], ct1[:],
                                        1.0 / (128.0 * 128.0 * 8.0), None,
                                        ALU.mult)
                ct3 = xpool.tile([P, 1], f32, name="ct3", tag="ct3")
                nc.vector.tensor_tensor(ct3[:], glv[:], bb[:, 2:3], ALU.mult)
                nc.vector.tensor_scalar(iCv[:], ct3[:], 1.0 / 128.0, None,
                                        ALU.mult)

            # ================= stage 3+4: projections + attention ============
            def agx(t, ic, sc, c0, cn):
                r0 = D * sc + P * ic
                return ag_out3[t][r0:r0 + P, c0:c0 + cn]

            # pools spanning attention + deferred stage-5 stats (s5* stay
            # open past attper for the tail)
            with tc.tile_pool(name="attper", bufs=1) as attper:
                qpT = [attper.tile([65, S], f32r, name=f"qpT{h}", tag=f"qpT{h}") for h in range(4)]
                kpT = [attper.tile([65, S], f32r, name=f"kpT{h}", tag=f"kpT{h}") for h in range(4)]
                qpA = [attper.tile([64, S], bf16, name=f"qpA{h}", tag=f"qpA{h}") for h in range(4)]
                kpA = [attper.tile([64, S], bf16, name=f"kpA{h}", tag=f"kpA{h}") for h in range(4)]
                vps = attper.tile([P, 16 * 260], bf16, name="vps", tag="vps")

                with tc.tile_pool(name="rhsp", bufs=4) as rhsp, \
                     tc.tile_pool(name="psp3", bufs=4, space="PSUM") as psp3:
                    for h in range(4):
                        nc.sync.dma_start(kpT[h][64:65, :], ones_row)
                    for t, wsb, dstT, dstA in ((0, wq_sb, qpT, qpA),
                                               (1, wk_sb, kpT, kpA)):
                        for sc in range(4):
                            rhs = [rhsp.tile([P, SH], bf16, name=f"rh{ic % 4}", tag=f"rh{ic % 4}")
                                   for ic in range(8)]
                            for ic in range(8):
                                r8 = rhsp.tile([P, SH], mybir.dt.int8,
                                               name=f"r8{ic % 4}",
                                               tag=f"r8{ic % 4}")
                                nc.sync.dma_start(r8[:],
                                                  agx(t, ic, sc, 0, SH))
                                nc.vector.tensor_copy(rhs[ic][:], r8[:])
                            for oh in range(2):
                                ps = psp3.tile([P, SH], f32, name="ps3", tag="ps3")
                                for ic in range(8):
                                    nc.tensor.matmul(
                                        ps[:], wsb[ic][:, oh * P:(oh + 1) * P],
                                        rhs[ic][:], start=(ic == 0),
                                        stop=(ic == 7))
                                hA, hB = oh * 2, oh * 2 + 1
                                nc.scalar.activation(
                                    dstT[hA][0:64, sc * SH:(sc + 1) * SH],
                                    ps[0:64, :], CPY)
                                nc.vector.tensor_copy(
                                    dstA[hA][0:64, sc * SH:(sc + 1) * SH],
                                    ps[0:64, :])
                                stg = rhsp.tile([P, SH], f32r, name="stg", tag="stg")
                                stgb = rhsp.tile([P, SH], bf16, name="stgb", tag="stgb")
                                nc.scalar.activation(stg[64:P, :],
                                                     ps[64:P, :], CPY)
                                nc.vector.tensor_copy(stgb[64:P, :], ps[64:P, :])
                                nc.sync.dma_start(
                                    dstT[hB][0:64, sc * SH:(sc + 1) * SH],
                                    stg[64:P, :])
                                nc.sync.dma_start(
                                    dstA[hB][0:64, sc * SH:(sc + 1) * SH],
                                    stgb[64:P, :])
                    for sc4 in range(4):
                      vstr = [rhsp.tile([P, SH], bf16, name=f"vs{ic % 4}",
                                        tag=f"vs{ic % 4}") for ic in range(8)]
                      for ic in range(8):
                          v8 = rhsp.tile([P, SH], mybir.dt.int8,
                                         name=f"v8{ic % 4}", tag=f"v8{ic % 4}")
                          nc.sync.dma_start(v8[:], agx(2, ic, sc4, 0, SH))
                          nc.vector.tensor_copy(vstr[ic][:], v8[:])
                      for qc in range(4):
                        kc = sc4 * 4 + qc
                        ps = psp3.tile([P, 256], f32, name="psv", tag="psv")
                        for ic in range(8):
                            nc.tensor.matmul(ps[:],
                                             vstr[ic][:, qc * P:(qc + 1) * P],
                                             wv_sb[ic][:],
                                             start=(ic == 0), stop=(ic == 7))
                        for h in range(4):
                            nc.scalar.activation(
                                vps[:, 260 * kc + 65 * h:260 * kc + 65 * h + 64],
                                ps[:, 64 * h:64 * h + 64], CPY)
                            nc.scalar.activation(
                                vps[:, 260 * kc + 65 * h + 64:
                                    260 * kc + 65 * h + 65],
                                ones128[:], CPY)

                # ---- attention: singles-granularity software pipeline ----
                # per kc step of block i the PE runs [A(i+2,kc), B(i,kc),
                # AV(i,kc-4)]; exp fires per B single; pass A runs two blocks
                # ahead so the -M row lands a full block before B reads it;
                # the epilogue of block i-1 (ACT copy, PE transposes, DVE
                # division) is stitched into block i's steps 0-7 so the PE
                # never drains at a block boundary (keeps the 2.4GHz p-state).
                # PSUM: psA 2x[128,1024] + psB 2x[128,512] + psX + psT = 8.
                with tc.tile_pool(name="mpool", bufs=3) as mpool, \
                     tc.tile_pool(name="ptp", bufs=8) as ptp, \
                     tc.tile_pool(name="trp", bufs=2) as trp, \
                     tc.tile_pool(name="xtsb", bufs=2) as xtsb, \
                     tc.tile_pool(name="psA", bufs=2, space="PSUM") as psA, \
                     tc.tile_pool(name="psB", bufs=2, space="PSUM") as psB, \
                     tc.tile_pool(name="psX", bufs=1, space="PSUM") as psX, \
                     tc.tile_pool(name="psT", bufs=1, space="PSUM") as psT:

                    blocks = [(h, qb) for qb in range(4) for h in range(4)]

                    mp_tiles = {}
                    xps_tiles = {}
                    pa_cur = [None]
                    pT_lists = {}

                    def new_xps(blk):
                        xps_tiles[blk] = psX.tile([65, SH], f32,
                                                  name="xps", tag="xps")

                    def emit_A_step(blk, s):
                        """one bf16 QK single [128 q x 512 k] into a bf16
                        PSUM pair-tile half; the row-max estimate is built
                        with a bf16 tensor-tensor max tree (2x DVE mode; a
                        plain PSUM f32 reduce would pace the whole pipeline
                        below the 2.4GHz PE p-state). bf16 rounding of the
                        logits costs <~25 on the estimate; pass B's exact
                        logits-minus-M keeps exp args bounded either way."""
                        h, qb = blk
                        q0 = qb * SH
                        qc, kb = s // 4, s % 4
                        if kb == 0:
                            mp_tiles[blk + (qc,)] = mpool.tile(
                                [P, 2], f32, name="Mp", tag="Mp")
                        if s % 2 == 0:
                            pa_cur[0] = psA.tile([P, 2 * SH], f32, name="pa",
                                                 tag="pa")
                        pa = pa_cur[0]
                        half = s % 2
                        nc.tensor.matmul(
                            pa[:, half * SH:(half + 1) * SH],
                            qpA[h][:, q0 + qc * P:q0 + (qc + 1) * P],
                            kpA[h][:, kb * SH:(kb + 1) * SH],
                            start=True, stop=True)
                        Mp = mp_tiles[blk + (qc,)]
                        if half == 1:
                            nc.vector.tensor_reduce(
                                Mp[:, kb // 2:kb // 2 + 1], pa[:], axis=AXX,
                                op=ALU.max)
                        if kb == 3:
                            ngm = mpool.tile([P, 1], f32r, name="ngm", tag="ngm")
                            nc.vector.tensor_reduce(ngm[:], Mp[:], axis=AXX,
                                                    op=ALU.max, negate=True)
                            nc.sync.dma_start(
                                qpT[h][64:65, q0 + qc * P:q0 + (qc + 1) * P],
                                ngm[:])
                            mp_tiles.pop(blk + (qc,))

                    def emit_B(blk, kc):
                        """one f32r QK single [128 k x 512 q] + its exp."""
                        h, qb = blk
                        q0 = qb * SH
                        pb = psB.tile([P, SH], f32, name="pb", tag="pb")
                        nc.tensor.matmul(
                            pb[:],
                            kpT[h][:, kc * P:(kc + 1) * P],
                            qpT[h][:, q0:q0 + SH],
                            start=True, stop=True)
                        pT = ptp.tile([P, SH], bf16, name="pT", tag="pT")
                        nc.scalar.activation(pT[:], pb[:], EXP, scale=Cq[:])
                        pT_lists[blk].append(pT)

                    def emit_AV(blk, kc):
                        h, qb = blk
                        nc.tensor.matmul(
                            xps_tiles[blk],
                            vps[:, 260 * kc + 65 * h:260 * kc + 65 * (h + 1)],
                            pT_lists[blk][kc],
                            start=(kc == 0), stop=(kc == 15))

                    def emit_epi_copy(blk):
                        """ACT copy drains xps -> SBUF, freeing the psX bank."""
                        xps = xps_tiles.pop(blk)
                        xt_s = xtsb.tile([65, SH], f32, name="xt_s", tag="xt_s")
                        nc.scalar.activation(xt_s[:], xps[:], CPY)
                        pT_lists.pop(blk)
                        return xt_s

                    def emit_epi_div(blk, xt_s, qc):
                        """transpose one 128-query chunk + divide by denom."""
                        h, qb = blk
                        ptx = psT.tile([P, 65], f32, name="ptx", tag="ptx")
                        nc.tensor.transpose(
                            ptx[:], xt_s[0:65, qc * P:(qc + 1) * P],
                            idf[0:65, 0:65])
                        rv = mpool.tile([P, 1], f32, name="rv", tag="rv")
                        rv0 = mpool.tile([P, 1], f32, name="rv0", tag="rv0")
                        nc.vector.reciprocal(rv0[:], ptx[:, 64:65])
                        nc.vector.tensor_tensor(rv[:], rv0[:], iCv[:],
                                                ALU.mult)
                        nc.vector.tensor_scalar(
                            xsb[qb * 4 + qc][:, 64 * h:64 * (h + 1)],
                            ptx[:, 0:64], rv[:], None, ALU.mult)

                    def emit_xag(qb):
                        for qc in range(4):
                            # scalar-triggered: keeps these off the sync
                            # stream, where pending -M DMAs would HOL-block
                            # them for more than a block
                            nc.scalar.dma_start(
                                xag_in[qb * SH + qc * P:
                                       qb * SH + (qc + 1) * P, :],
                                xsb[qb * 4 + qc][:])
                        nc.gpsimd.collective_compute(
                            "AllGather", ALU.bypass, replica_groups=groups4,
                            ins=[xag_in[qb * SH:(qb + 1) * SH, :].opt()],
                            outs=[xag_out4[qb].opt()])

                    # prologue: pass A for blocks 0 and 1
                    for s in range(16):
                        emit_A_step(blocks[0], s)
                    for s in range(16):
                        emit_A_step(blocks[1], s)

                    for i, blk in enumerate(blocks):
                        h, qb = blk
                        prv = blocks[i - 1] if i > 0 else None
                        nxt2 = blocks[i + 2] if i + 2 < len(blocks) else None
                        new_xps(blk)
                        pT_lists[blk] = []
                        xt_prev = [None]
                        for kc in range(16):
                            if prv is not None and kc < 4:
                                emit_AV(prv, 12 + kc)
                                if kc == 3:
                                    xt_prev[0] = emit_epi_copy(prv)
                            emit_B(blk, kc)
                            if nxt2 is not None:
                                emit_A_step(nxt2, kc)
                            if 4 <= kc < 8 and prv is not None:
                                emit_epi_div(prv, xt_prev[0], kc - 4)
                                if kc == 7 and prv[0] == 3:
                                    # previous qb group complete: ship its x
                                    emit_xag(prv[1])
                            if kc >= 4:
                                emit_AV(blk, kc - 4)
                        # spread deferred x-stats one tile per block so the
                        # DVE never pushes the block pace above the PE's;
                        # late blocks take a second tile to shorten the tail
                        if i >= 6:
                            emit_x_stat_tile(i - 6, mpool)

                    # flush: last block's AV tail + epilogue + its qb AG
                    lst = blocks[-1]
                    for kc in range(12, 16):
                        emit_AV(lst, kc)
                    xt_l = emit_epi_copy(lst)
                    for qc in range(4):
                        emit_epi_div(lst, xt_l, qc)
                    emit_xag(lst[1])

                if DEBUG:
                    with tc.tile_pool(name="dbgp", bufs=1) as dbgp:
                        nc.sync.dma_start(dbg_qpa, qpA[0][:, 0:256])
                        nc.sync.dma_start(dbg_kpa, kpA[0][:, 0:256])
                        nc.sync.dma_start(dbg_qpt, qpT[0][:, 0:256])
                        nc.sync.dma_start(dbg_kpt, kpT[0][:, 0:256])
                        nc.sync.dma_start(dbg_vps, vps[:, 0:260])
                        nc.sync.dma_start(dbg_xsb, xsb[0][:])

            # ====== stage 5 tail: last stats, AR2, quant, projection =====
            with tc.tile_pool(name="fpool", bufs=1) as fpool, \
                 tc.tile_pool(name="f2pool", bufs=3) as f2pool, \
                 tc.tile_pool(name="psf", bufs=1, space="PSUM") as psf:
                for rc in range(10, 16):
                    emit_x_stat_tile(rc, f2pool)
                xq0T = fpool.tile([P, 16 * D], bf16, name="xq0T", tag="xq0T")
                Am21 = fpool.tile([P, 1], f32, name="Am21", tag="Am21")
                nc.vector.tensor_reduce(Am21[:], Am2[:], axis=AXX, op=ALU.max)
                stage2 = fpool.tile([1, 8], f32, name="stage2", tag="stage2")
                nc.vector.memset(stage2[:], 0.0)
                tree_max(f2pool, psf, Am21, 1, stage2)
                nc.sync.dma_start(ar2_in[:], stage2[:])
                nc.gpsimd.collective_compute(
                    "AllReduce", ALU.max, replica_groups=groups8,
                    ins=[ar2_in.opt()], outs=[ar2_out.opt()])
                arow2 = fpool.tile([1, 8], f32, name="arow2", tag="arow2")
                nc.sync.dma_start(arow2[:], ar2_out[:])
                psb3 = psf.tile([P, 8], f32, name="psb3", tag="psb3")
                nc.tensor.matmul(psb3[:], ones1[:], arow2[:], start=True,
                                 stop=True)
                G2 = fpool.tile([P, 8], f32, name="G2", tag="G2")
                nc.vector.tensor_copy(G2[:], psb3[:])
                s128b = fpool.tile([P, 1], f32, name="s128b", tag="s128b")
                ig0, gl0 = rsqrt_nr(fpool, G2[:, 0:1], "f0")
                nc.vector.tensor_scalar(s128b[:], ig0[:], 128.0, None,
                                        ALU.mult)
                bb2 = fpool.tile([P, 8], f32, name="bb2", tag="bb2")
                psb4 = psf.tile([P, 8], f32, name="psb4", tag="psb3")
                nc.tensor.matmul(psb4[:], ones1[:], beta_sb[:], start=True,
                                 stop=True)
                nc.vector.tensor_copy(bb2[:], psb4[:])
                C0 = fpool.tile([P, 1], f32, name="C0", tag="C0")
                nc.vector.tensor_tensor(C0[:], gl0[:], bb2[:, 3:4], ALU.mult)
                nc.vector.tensor_scalar(C0[:], C0[:], 1.0 / 128.0, None,
                                        ALU.mult)
                # quant + flipped output projection, interleaved per 4-row
                # group so the PE matmuls overlap the next group's quant
                xq4 = xq0T[:].rearrange("p (rc ic c) -> p rc ic c",
                                        rc=16, ic=8, c=P)
                for r4 in range(4):
                    for rc in range(4 * r4, 4 * r4 + 4):
                        xf = f2pool.tile([P, D], bf16, name="xf2", tag="xf2")
                        qb = rc // 4
                        for j in range(4):
                            nc.sync.dma_start(
                                xf[:, 256 * j:256 * (j + 1)],
                                xag_out4[qb][SH * j + (rc % 4) * P:
                                             SH * j + (rc % 4 + 1) * P, :])
                        quant_tile(f2pool, psf, xf, rst2[rc], nmu2[rc],
                                   s128b, xq0T[:, rc * D:(rc + 1) * D],
                                   f"f{rc}")
                    for oh in range(2):
                        ps = psf.tile([P, SH], f32, name="pso", tag="pso",
                                      bufs=2)
                        for ic in range(8):
                            nc.tensor.matmul(
                                ps[:],
                                w0_sb[ic][:, oh * P:(oh + 1) * P],
                                xq4[:, 4 * r4:4 * (r4 + 1), ic, :],
                                start=(ic == 0), stop=(ic == 7))
                        yt = f2pool.tile([P, SH], f32, name="yt", tag="yt")
                        nc.scalar.activation(yt[:], ps[:], CPY, scale=C0[:])
                        nc.sync.dma_start(
                            y[oh * P:(oh + 1) * P, r4 * SH:(r4 + 1) * SH],
                            yt[:])

    nc.compile()
    return nc


def _prep_host(inputs):
    import ml_dtypes
    bf = ml_dtypes.bfloat16
    ws = [inputs["wq_w"], inputs["wk_w"], inputs["wv_w"], inputs["w0_w"]]
    signs = []
    betas = np.zeros((1, 8), np.float32)
    for i, w in enumerate(ws):
        w64 = np.asarray(w, np.float64)
        signs.append(np.sign(w64 - w64.mean()).astype(np.float32))
        betas[0, i] = np.abs(w64).mean()
    id_bf = np.eye(128, dtype=bf)
    id_f = np.eye(128, dtype=np.float32)
    ones_row = np.ones((1, S), np.float32)
    qf = np.asarray(inputs["q"], np.float32).reshape(2 * S, D)
    kf = np.asarray(inputs["k"], np.float32).reshape(2 * S, D)
    vf = np.asarray(inputs["v"], np.float32).reshape(2 * S, D)
    in_maps = []
    for c in range(8):
        b, g = c // 4, c % 4
        r0 = b * S + g * SH
        m = {
            "x0": np.ascontiguousarray(qf[r0:r0 + SH]),
            "x1": np.ascontiguousarray(kf[r0:r0 + SH]),
            "x2": np.ascontiguousarray(vf[r0:r0 + SH]),
            "wo": np.ascontiguousarray(
                signs[3].T[:, 256 * g:256 * (g + 1)]).astype(bf),
            "idbf": id_bf, "idf": id_f,
            "onesrow": ones_row, "betas": betas,
        }
        for t in range(3):
            m[f"w{t}"] = np.ascontiguousarray(
                signs[t].T[:, 256 * g:256 * (g + 1)]).astype(bf)
        in_maps.append(m)
    return in_maps


def _run(inputs, trace=False):
    global _COMPILED
    from concourse import bass_utils
    if _COMPILED is None:
        _COMPILED = _build()
    nc = _COMPILED
    in_maps = _prep_host(inputs)
    res = bass_utils.run_bass_kernel_spmd(nc, in_maps, core_ids=list(range(8)),
                                          trace=trace)
    out = np.zeros((B, S, D), np.float32)
    for c in range(8):
        b, g = c // 4, c % 4
        out[b, :, 256 * g:256 * (g + 1)] = res.results[c]["y"].T
    return out, res


def kernel(**inputs):
    mask = np.asarray(inputs["mask"])
    if not (mask == 1).all():
        return _numpy_fallback(**inputs)
    out, _ = _run(inputs, trace=False)
    return out


def _numpy_fallback(q, k, v, mask, wq_w, wk_w, wv_w, w0_w):
    f = np.float32

    def ln(x):
        mu = x.mean(-1, keepdims=True, dtype=f)
        var = np.mean((x - mu) ** 2, -1, keepdims=True, dtype=f)
        return ((x - mu) / np.sqrt(var + f(LN_EPS))).astype(f)

    def bitlin(x, w):
        xn = ln(np.asarray(x, f))
        mx = np.abs(xn).max()
        xq = np.round(xn * (f(128.0) / mx)).astype(f)
        xq = (np.mod(xq + 128.0, 256.0) - 128.0).astype(f)
        wq = np.sign(w - w.mean(dtype=f)).astype(f)
        beta = np.abs(w).mean(dtype=f)
        return ((xq @ wq.T) * f(mx / 128 * beta)).astype(f)

    qp = bitlin(q, wq_w).reshape(B, S, H, DK).transpose(0, 2, 1, 3)
    kp = bitlin(k, wk_w).reshape(B, S, H, DK).transpose(0, 2, 1, 3)
    vp = bitlin(v, wv_w).reshape(B, S, H, DK).transpose(0, 2, 1, 3)
    out = np.zeros((B, H, S, DK), f)
    mask = np.asarray(mask)
    for b in range(B):
        for h in range(H):
            att = (qp[b, h] @ kp[b, h].T) / f(np.sqrt(DK))
            att = np.where(mask[b] == 0, f(-1e9), att).astype(f)
            att = att - att.max(-1, keepdims=True)
            e = np.exp(att)
            p = e / e.sum(-1, keepdims=True)
            out[b, h] = p @ vp[b, h]
    x = out.transpose(0, 2, 1, 3).reshape(B, S, H * DK)
    return bitlin(x, w0_w)



# revision 52
# speedup vs baseline: 1.2035x; 1.0951x over previous
"""BitLinear multi-head attention on 8 trn2 NeuronCores.

Sharding: core c handles batch b=c//4 and head group g=c%4 (heads 4g..4g+3).
Stages:
  0. dummy collective at t=0 absorbs the first-collective barrier under the
     input loads + LN stats of the core's 512-row shard of q/k/v
  1. AllReduce(max) of the 3 global absmax scalars
  2. per tensor: quantize to int8-valued bf16, PE-transpose, AllGather
     (three pipelined collectives so projections overlap the later gathers)
  3. projections qpT/kpT (transposed, integer-exact bf16 matmuls) and vp
     (+ bf16 shadow copies qpA/kpA for the max-estimate pass)
  4. attention blocks (h inner, qb outer), software-pipelined; per block the
     PE runs dtype-uniform batches to avoid mode-switch stalls:
       [16 f32r QK matmuls (pass B, paired into [128,1024] PSUM) + 8 exps]
       [16 bf16 att@V matmuls] [16 bf16 pass-A matmuls of the NEXT block]
     pass A estimates the per-query max in bf16 (exp arg stays <= ~+7, safe);
     DVE negated-max reduces write -M via DMA into qpT's augmented row; pass
     B's ones-row/-M-row trick yields exact logits-minus-max; the ones column
     of vps gives the softmax denominator.  After each qb group finishes, its
     x slice is AllGathered (hidden under the next group's compute) and its
     LN stats are emitted one group deferred.
  5. tail: AR2 (global absmax), streamed re-quantization of x, output
     projection (feature-sharded); host assembles the final tensor.
"""
import numpy as np

B, S, D, H, DK = 2, 2048, 1024, 16, 64
SH = 512            # rows per core shard (stage 0)
P = 128
MAGIC = 12582912.0  # 1.5 * 2**23, forces RNE-round-to-int for |x| < 2**22
LN_EPS = 1e-5

_COMPILED = None
DEBUG = False


def _build():
    import concourse.tile as tile
    from concourse import bacc, mybir

    f32 = mybir.dt.float32
    f32r = mybir.dt.float32r
    bf16 = mybir.dt.bfloat16
    EXP = mybir.ActivationFunctionType.Exp
    CPY = mybir.ActivationFunctionType.Copy
    IDN = mybir.ActivationFunctionType.Identity
    SQ = mybir.ActivationFunctionType.Square
    SQRT = mybir.ActivationFunctionType.Sqrt
    ALU = mybir.AluOpType
    AXX = mybir.AxisListType.X

    nc = bacc.Bacc("TRN2", target_bir_lowering=False, debug=False,
                   enable_asserts=False, num_devices=8)

    x_sh = [nc.dram_tensor(f"x{t}", [SH, D], f32, kind="ExternalInput").ap()
            for t in range(3)]
    w_t = [nc.dram_tensor(f"w{t}", [D, 256], bf16, kind="ExternalInput").ap()
           for t in range(3)]                       # sign(w)^T o-slices, bf16
    w0_t = nc.dram_tensor("wo", [D, 256], bf16, kind="ExternalInput").ap()
    id_bf = nc.dram_tensor("idbf", [P, P], bf16, kind="ExternalInput").ap()
    id_f = nc.dram_tensor("idf", [P, P], f32, kind="ExternalInput").ap()
    ones_row = nc.dram_tensor("onesrow", [1, S], f32r, kind="ExternalInput").ap()
    betas = nc.dram_tensor("betas", [1, 8], f32, kind="ExternalInput").ap()
    y = nc.dram_tensor("y", [256, S], f32, kind="ExternalOutput").ap()
    if DEBUG:
        dbg_qpa = nc.dram_tensor("dbg_qpa", [64, 256], bf16, kind="ExternalOutput").ap()
        dbg_kpa = nc.dram_tensor("dbg_kpa", [64, 256], bf16, kind="ExternalOutput").ap()
        dbg_qpt = nc.dram_tensor("dbg_qpt", [65, 256], f32r, kind="ExternalOutput").ap()
        dbg_kpt = nc.dram_tensor("dbg_kpt", [65, 256], f32r, kind="ExternalOutput").ap()
        dbg_vps = nc.dram_tensor("dbg_vps", [P, 260], bf16, kind="ExternalOutput").ap()
        dbg_xsb = nc.dram_tensor("dbg_xsb", [P, 256], f32, kind="ExternalOutput").ap()

    groups8 = [list(range(8))]
    groups4 = [[0, 1, 2, 3], [4, 5, 6, 7]]

    with tile.TileContext(nc) as tc:
        with tc.tile_pool(name="dram", bufs=1, space="DRAM") as dram, \
             tc.tile_pool(name="persist", bufs=1) as per:

            # ---- always-live SBUF ----
            xsb = [per.tile([P, 256], bf16, name=f"xsb{i}", tag=f"xsb{i}") for i in range(16)]
            Gb = per.tile([P, 8], f32, name="Gb", tag="Gb")
            Cq = per.tile([P, 1], f32, name="Cq", tag="Cq")
            iCv = per.tile([P, 1], f32, name="iCv", tag="iCv")
            idb = per.tile([P, P], bf16, name="idb", tag="idb")
            idf = per.tile([P, P], f32, name="idf", tag="idf")
            beta_sb = per.tile([1, 8], f32, name="beta_sb", tag="beta_sb")
            ones1 = per.tile([1, P], f32, name="ones1", tag="ones1")
            ones128 = per.tile([P, 1], f32, name="ones128", tag="ones128")
            wq_sb = [per.tile([P, 256], bf16, name=f"wq{ic}", tag=f"wq{ic}")
                     for ic in range(8)]
            wk_sb = [per.tile([P, 256], bf16, name=f"wk{ic}", tag=f"wk{ic}")
                     for ic in range(8)]
            wv_sb = [per.tile([P, 256], bf16, name=f"wv{ic}", tag=f"wv{ic}")
                     for ic in range(8)]
            w0_sb = [per.tile([P, 256], bf16, name=f"w0{ic}", tag=f"w0{ic}")
                     for ic in range(8)]
            # dependency-free dummy collective fired before anything else:
            # the first collective pays the cross-core rendezvous barrier
            # (39-110us of launch skew) — absorb it under the input loads
            # and LN stats instead of under the q-absmax AllReduce
            dmy_in = dram.tile([1, 8], f32, name="dmy_in", tag="dmy_in")
            dmy_out = dram.tile([1, 8], f32, name="dmy_out", tag="dmy_out")
            nc.gpsimd.collective_compute(
                "AllReduce", ALU.max, replica_groups=groups8,
                ins=[dmy_in.opt()], outs=[dmy_out.opt()])
            nc.sync.dma_start(idb[:], id_bf)
            nc.sync.dma_start(idf[:], id_f)
            nc.sync.dma_start(beta_sb[:], betas)
            nc.vector.memset(ones1[:], 1.0)
            nc.vector.memset(ones128[:], 1.0)

            def emit_weight_loads():
                # PE-triggered: the tensor engine is idle until projections,
                # so 8MB of weight traffic never delays the sync queue's
                # stage/ar1 DMAs (whose latency gates the absmax AllReduces)
                for ic in range(8):
                    nc.gpsimd.dma_start(wq_sb[ic][:],
                                        w_t[0][ic * P:(ic + 1) * P, :])
                    nc.gpsimd.dma_start(wk_sb[ic][:],
                                        w_t[1][ic * P:(ic + 1) * P, :])
                    nc.gpsimd.dma_start(wv_sb[ic][:],
                                        w_t[2][ic * P:(ic + 1) * P, :])
                    nc.gpsimd.dma_start(w0_sb[ic][:],
                                        w0_t[ic * P:(ic + 1) * P, :])

            # DRAM bounce buffers
            i8 = mybir.dt.int8
            ag_in = dram.tile([3 * D, SH], i8, name="ag_in", tag="ag_in")
            ag_out3 = [dram.tile([4 * D, SH], i8, name=f"ago{t}", tag=f"ago{t}")
                       for t in range(3)]
            ar1_in = dram.tile([1, 8], f32, name="ar1_in", tag="ar1_in")
            ar1_out = dram.tile([1, 8], f32, name="ar1_out", tag="ar1_out")
            ar2_in = dram.tile([1, 8], f32, name="ar2_in", tag="ar2_in")
            ar2_out = dram.tile([1, 8], f32, name="ar2_out", tag="ar2_out")
            xag_in = dram.tile([S, 256], bf16, name="xag_in", tag="xag_in")
            xag_out4 = [dram.tile([4 * SH, 256], bf16, name=f"xao{j}",
                                  tag=f"xao{j}") for j in range(4)]


            def ln_stats_tile(pool, xtile, scr, am_out, sfx):
                """LN stats for one [P, D] tile. Returns (rstd, negmu) tiles."""
                sm = pool.tile([P, 1], f32, name="sm", tag="sm")
                nmu = pool.tile([P, 1], f32, name=f"nmu{sfx}", tag=f"nmu{sfx}")
                ssq = pool.tile([P, 1], f32, name="ssq", tag="ssq")
                rst = pool.tile([P, 1], f32, name=f"rst{sfx}", tag=f"rst{sfx}")
                mxs = pool.tile([P, 1], f32, name="mxs", tag="mxs")
                r2 = pool.tile([P, 1], f32, name="r2", tag="r2")
                nc.vector.tensor_reduce(sm[:], xtile[:], axis=AXX, op=ALU.add)
                nc.scalar.activation(nmu[:], sm[:], CPY, scale=-1.0 / D)
                nc.scalar.activation(scr[:], xtile[:], SQ, bias=nmu[:],
                                     accum_out=ssq[:])
                var_ = pool.tile([P, 1], f32, name="var_", tag="var_")
                rvar = pool.tile([P, 1], f32, name="rvar", tag="rvar")
                y0 = pool.tile([P, 1], f32, name="y0", tag="y0")
                nr = pool.tile([P, 1], f32, name="nr", tag="nr")
                nc.scalar.activation(var_[:], ssq[:], CPY, scale=1.0 / D,
                                     bias=LN_EPS)
                nc.vector.reciprocal(rvar[:], var_[:])
                nc.scalar.activation(y0[:], rvar[:], SQRT)
                # two Newton steps: rstd = y0*(1.5 - 0.5*var*y0^2)
                nc.vector.tensor_tensor(nr[:], y0[:], y0[:], ALU.mult)
                nc.vector.tensor_tensor(nr[:], nr[:], var_[:], ALU.mult)
                nc.vector.tensor_scalar(nr[:], nr[:], -0.5, 1.5, ALU.mult,
                                        ALU.add)
                nc.vector.tensor_tensor(y0[:], y0[:], nr[:], ALU.mult)
                nc.vector.tensor_tensor(nr[:], y0[:], y0[:], ALU.mult)
                nc.vector.tensor_tensor(nr[:], nr[:], var_[:], ALU.mult)
                nc.vector.tensor_scalar(nr[:], nr[:], -0.5, 1.5, ALU.mult,
                                        ALU.add)
                nc.vector.tensor_tensor(rst[:], y0[:], nr[:], ALU.mult)
                nc.vector.tensor_reduce(mxs[:], scr[:], axis=AXX, op=ALU.max)
                nc.vector.tensor_tensor(r2[:], rst[:], rst[:], ALU.mult)
                # squared row amax: max((x-mu)^2) * rstd^2  (no sqrt here)
                nc.vector.tensor_tensor(am_out, mxs[:], r2[:], ALU.mult)
                return rst, nmu

            def tree_max(pool, pspool, src, ncols, out_row):
                """Partition-axis max over src[:, :ncols] -> out_row[0:1, :ncols]."""
                ptr = pspool.tile([P, P], f32, name="trps", tag="trps")
                nc.tensor.transpose(ptr[0:ncols, :], src[:, :ncols], idf[:])
                a = pool.tile([P, P], f32, name="trA", tag="trA")
                nc.scalar.activation(a[0:ncols, :], ptr[0:ncols, :], CPY)
                m = pool.tile([P, 1], f32, name="trM", tag="trM")
                nc.vector.tensor_reduce(m[0:ncols, :], a[0:ncols, :], axis=AXX,
                                        op=ALU.max)
                nc.sync.dma_start(out_row[0:1, 0:ncols], m[0:ncols, :])

            def rsqrt_nr(pool, gsq_ap, sfx):
                """accurate rsqrt of a [P,1] squared-max -> (1/g, g) tiles."""
                rv_ = pool.tile([P, 1], f32, name=f"rv{sfx}", tag=f"rv{sfx}")
                yy = pool.tile([P, 1], f32, name=f"yy{sfx}", tag=f"yy{sfx}")
                tn = pool.tile([P, 1], f32, name=f"tn{sfx}", tag=f"tn{sfx}")
                ig = pool.tile([P, 1], f32, name=f"ig{sfx}", tag=f"ig{sfx}")
                gl = pool.tile([P, 1], f32, name=f"gl{sfx}", tag=f"gl{sfx}")
                nc.vector.reciprocal(rv_[:], gsq_ap)
                nc.scalar.activation(yy[:], rv_[:], SQRT)
                nc.vector.tensor_tensor(tn[:], yy[:], yy[:], ALU.mult)
                nc.vector.tensor_tensor(tn[:], tn[:], gsq_ap, ALU.mult)
                nc.vector.tensor_scalar(tn[:], tn[:], -0.5, 1.5, ALU.mult,
                                        ALU.add)
                nc.vector.tensor_tensor(ig[:], yy[:], tn[:], ALU.mult)
                nc.vector.tensor_tensor(gl[:], gsq_ap, ig[:], ALU.mult)
                return ig, gl

            def quant_tile(pool, pspool, xtile, rst, nmu, s128, dst_tiles, sfx):
                """Quantize one [P, D] tile -> 8 transposed [P, P] bf16 writes."""
                sc_ = pool.tile([P, 1], f32, name="sc_", tag="sc_")
                bp = pool.tile([P, 1], f32, name="bp", tag="bp")
                nc.vector.tensor_tensor(sc_[:], rst[:], s128[:], ALU.mult)
                nc.vector.tensor_tensor(bp[:], nmu[:], sc_[:], ALU.mult)
                # NOTE: MAGIC must NOT fold into the ACT bias — ACT's fused
                # multiply-add rounds once, flipping ints vs the reference's
                # two-step f32 rounding (verified on HW).
                t1a = pool.tile([P, D], f32, name="t1a", tag="t1a")
                t2 = pool.tile([P, D], f32, name="t2", tag="t2")
                msk = pool.tile([P, D], f32, name="msk", tag="msk")
                xqb = pool.tile([P, D], bf16, name="xqb", tag="xqb")
                nc.scalar.activation(t1a[:], xtile[:], IDN, scale=sc_[:], bias=bp[:])
                nc.scalar.activation(t2[:], t1a[:], CPY, bias=MAGIC)
                nc.vector.tensor_scalar(msk[:], t2[:], MAGIC + 127.5, 256.0,
                                        ALU.is_ge, ALU.mult)
                nc.vector.scalar_tensor_tensor(xqb[:], t2[:], -MAGIC, msk[:],
                                               ALU.add, ALU.subtract)
                pstr = pspool.tile([P, D], bf16, name="pstr", tag="pstr",
                                   bufs=2)
                for ic in range(8):
                    nc.tensor.transpose(pstr[:, ic * P:(ic + 1) * P],
                                        xqb[:, ic * P:(ic + 1) * P],
                                        idb[:])
                if dst_tiles is not None and not hasattr(dst_tiles, "ap"):
                    # DRAM target: one wide PSUM->SBUF cast to int8 (halves
                    # the AllGather payload), then ONE 3D-AP DMA — 8 small
                    # DMAs each cost ~600ns of sync-stream time and would
                    # pace the AllGather triggers
                    t_, rc_ = dst_tiles
                    stg = pool.tile([P, D], mybir.dt.int8, name="qstg",
                                    tag="qstg")
                    nc.vector.tensor_copy(stg[:], pstr[:])
                    for ic in range(8):
                        nc.sync.dma_start(
                            ag_in[t_ * D + ic * P:t_ * D + (ic + 1) * P,
                                  rc_ * P:(rc_ + 1) * P],
                            stg[:, ic * P:(ic + 1) * P])
                else:
                    # one wide SBUF target: single DVE copy
                    nc.vector.tensor_copy(dst_tiles, pstr[:])

            Am2 = per.tile([P, 16], f32, name="Am2", tag="Am2")
            scr2 = per.tile([P, D], f32, name="scr2", tag="scr2")
            rst2 = [per.tile([P, 1], f32, name=f"rk{rc}", tag=f"rk{rc}")
                    for rc in range(16)]
            nmu2 = [per.tile([P, 1], f32, name=f"nk{rc}", tag=f"nk{rc}")
                    for rc in range(16)]

            def emit_x_stat_tile(rc, pool):
                """deferred stage-5 LN stats for one 128-row chunk of x."""
                qb = rc // 4
                xf = pool.tile([P, D], bf16, name="xf", tag="xf")
                for j in range(4):
                    # gpsimd SWDGE: a wait on the AG here must not HOL-block
                    # the sync queue that carries the -M row DMAs
                    nc.gpsimd.dma_start(
                        xf[:, 256 * j:256 * (j + 1)],
                        xag_out4[qb][SH * j + (rc % 4) * P:
                                     SH * j + (rc % 4 + 1) * P, :])
                r_, n_ = ln_stats_tile(pool, xf, scr2,
                                       Am2[:, rc:rc + 1], "s5")
                nc.vector.tensor_copy(rst2[rc][:], r_[:])
                nc.vector.tensor_copy(nmu2[rc][:], n_[:])

            # ================= stage 0-2: stats, AR, quantize, AGs ===========
            with tc.tile_pool(name="xpool", bufs=1) as xpool, \
                 tc.tile_pool(name="spool", bufs=4) as spool, \
                 tc.tile_pool(name="qpool", bufs=3) as qpool, \
                 tc.tile_pool(name="pst0", bufs=2, space="PSUM") as pst0:
                scr = spool.tile([P, D], f32, name="scr", tag="scr")
                Am = xpool.tile([P, 16], f32, name="Am", tag="Am")
                xt_all, rst_all, nmu_all = [], [], []
                for t in range(3):
                    xts = []
                    for rc in range(4):
                        xt = xpool.tile([P, D], f32, name=f"x{t}_{rc}", tag=f"x{t}_{rc}")
                        nc.sync.dma_start(xt[:], x_sh[t][rc * P:(rc + 1) * P, :])
                        xts.append(xt)
                    rs, nm = [], []
                    for rc in range(4):
                        r_, n_ = ln_stats_tile(spool, xts[rc], scr,
                                               Am[:, 4 * t + rc:4 * t + rc + 1],
                                               f"s0_{t}_{rc}")
                        rs.append(r_)
                        nm.append(n_)
                    xt_all.append(xts)
                    rst_all.append(rs)
                    nmu_all.append(nm)
                    if t == 2:
                        emit_weight_loads()
                # per-tensor max -> its own tiny AllReduce, so quantization
                # of q starts without waiting for k/v stats
                Am3 = xpool.tile([P, 3], f32, name="Am3", tag="Am3")
                arow = xpool.tile([1, 8], f32, name="arow", tag="arow")
                for t in range(3):
                    nc.vector.tensor_reduce(Am3[:, t:t + 1],
                                            Am[:, 4 * t:4 * t + 4],
                                            axis=AXX, op=ALU.max)
                    stage = xpool.tile([1, 8], f32, name=f"stage{t}",
                                       tag=f"stage{t}")
                    nc.vector.memset(stage[:], 0.0)
                    tree_max(spool, pst0, Am3[:, t:t + 1], 1, stage)
                    nc.sync.dma_start(ar1_in[0:1, t:t + 1], stage[0:1, 0:1])
                    nc.gpsimd.collective_compute(
                        "AllReduce", ALU.max, replica_groups=groups8,
                        ins=[ar1_in[0:1, t:t + 1].opt()],
                        outs=[ar1_out[0:1, t:t + 1].opt()])
                    # gpsimd SWDGE: this read waits on the AllReduce, and on
                    # a sync hw queue that wait poisons every DMA behind it
                    nc.gpsimd.dma_start(arow[0:1, t:t + 1],
                                        ar1_out[0:1, t:t + 1])
                for t in range(3):
                    psb = pst0.tile([P, 1], f32, name="psb", tag="psb")
                    nc.tensor.matmul(psb[:], ones1[:], arow[:, t:t + 1],
                                     start=True, stop=True)
                    nc.vector.tensor_copy(Gb[:, t:t + 1], psb[:])
                    s128 = qpool.tile([P, 1], f32, name="s128", tag="s128")
                    ig_t, _ = rsqrt_nr(qpool, Gb[:, t:t + 1], f"q{t}")
                    nc.vector.tensor_scalar(s128[:], ig_t[:], 128.0, None,
                                            ALU.mult)
                    for rc in range(4):
                        quant_tile(qpool, pst0, xt_all[t][rc], rst_all[t][rc],
                                   nmu_all[t][rc], s128, (t, rc), f"q{t}{rc}")
                    nc.gpsimd.collective_compute(
                        "AllGather", ALU.bypass, replica_groups=groups4,
                        ins=[ag_in[t * D:(t + 1) * D, :].opt()],
                        outs=[ag_out3[t].opt()])

                # exp scale C = gq*gk*bq*bk/(128*128*8);  iCv = 128/(gv*bv)
                bb = xpool.tile([P, 8], f32, name="bb", tag="bb")
                psb2 = pst0.tile([P, 8], f32, name="psb2", tag="psb2")
                nc.tensor.matmul(psb2[:], ones1[:], beta_sb[:], start=True,
                                 stop=True)
                nc.vector.tensor_copy(bb[:], psb2[:])
                ct1 = xpool.tile([P, 1], f32, name="ct1", tag="ct1")
                ct2 = xpool.tile([P, 1], f32, name="ct2", tag="ct2")
                _, glq = rsqrt_nr(xpool, Gb[:, 0:1], "lq")
                _, glk = rsqrt_nr(xpool, Gb[:, 1:2], "lk")
                _, glv = rsqrt_nr(xpool, Gb[:, 2:3], "lv")
                nc.vector.tensor_tensor(ct1[:], glq[:], glk[:], ALU.mult)
                nc.vector.tensor_tensor(ct2[:], bb[:, 0:1], bb[:, 1:2], ALU.mult)
                nc.vector.tensor_tensor(ct1[:], ct1[:], ct2[:], ALU.mult)
                nc.vector.tensor_scalar(Cq[:], ct1[:],
                                        1.0 / (128.0 * 128.0 * 8.0), None,
                                        ALU.mult)
                ct3 = xpool.tile([P, 1], f32, name="ct3", tag="ct3")
                nc.vector.tensor_tensor(ct3[:], glv[:], bb[:, 2:3], ALU.mult)
                nc.vector.tensor_scalar(iCv[:], ct3[:], 1.0 / 128.0, None,
                                        ALU.mult)

            # ================= stage 3+4: projections + attention ============
            def agx(t, ic, sc, c0, cn):
                r0 = D * sc + P * ic
                return ag_out3[t][r0:r0 + P, c0:c0 + cn]

            # pools spanning attention + deferred stage-5 stats (s5* stay
            # open past attper for the tail)
            with tc.tile_pool(name="attper", bufs=1) as attper:
                qpT = [attper.tile([65, S], f32r, name=f"qpT{h}", tag=f"qpT{h}") for h in range(4)]
                kpT = [attper.tile([65, S], f32r, name=f"kpT{h}", tag=f"kpT{h}") for h in range(4)]
                qpA = [attper.tile([64, S], bf16, name=f"qpA{h}", tag=f"qpA{h}") for h in range(4)]
                kpA = [attper.tile([64, S], bf16, name=f"kpA{h}", tag=f"kpA{h}") for h in range(4)]
                vps = attper.tile([P, 16 * 260], bf16, name="vps", tag="vps")
                # denominator ones-columns (col 64 of every 65-wide head
                # slice) in one strided memset instead of 64 ACT copies
                nc.vector.memset(
                    vps[:].rearrange("p (j c) -> p j c", j=64, c=65)[:, :, 64:65],
                    1.0)

                with tc.tile_pool(name="rhsp", bufs=4) as rhsp, \
                     tc.tile_pool(name="r8p", bufs=2) as r8p, \
                     tc.tile_pool(name="psp3", bufs=4, space="PSUM") as psp3:
                    for h in range(4):
                        nc.sync.dma_start(kpT[h][64:65, :], ones_row)
                    for t, wsb, dstT, dstA in ((0, wq_sb, qpT, qpA),
                                               (1, wk_sb, kpT, kpA)):
                        for sc in range(4):
                            rhs = [rhsp.tile([P, SH], bf16, name=f"rh{ic % 4}", tag=f"rh{ic % 4}")
                                   for ic in range(8)]
                            r8 = r8p.tile([P, 8 * SH], mybir.dt.int8,
                                          name="r8", tag="r8")
                            nc.sync.dma_start(
                                r8[:].rearrange("p (ic c) -> p ic c", ic=8),
                                ag_out3[t][D * sc:D * (sc + 1), :].rearrange(
                                    "(ic p) c -> p ic c", ic=8))
                            for ic in range(8):
                                nc.vector.tensor_copy(
                                    rhs[ic][:], r8[:, ic * SH:(ic + 1) * SH])
                            for oh in range(2):
                                ps = psp3.tile([P, SH], f32, name="ps3", tag="ps3")
                                for ic in range(8):
                                    nc.tensor.matmul(
                                        ps[:], wsb[ic][:, oh * P:(oh + 1) * P],
                                        rhs[ic][:], start=(ic == 0),
                                        stop=(ic == 7))
                                hA, hB = oh * 2, oh * 2 + 1
                                nc.scalar.activation(
                                    dstT[hA][0:64, sc * SH:(sc + 1) * SH],
                                    ps[0:64, :], CPY)
                                nc.vector.tensor_copy(
                                    dstA[hA][0:64, sc * SH:(sc + 1) * SH],
                                    ps[0:64, :])
                                stg = rhsp.tile([P, SH], f32r, name="stg", tag="stg")
                                stgb = rhsp.tile([P, SH], bf16, name="stgb", tag="stgb")
                                nc.scalar.activation(stg[64:P, :],
                                                     ps[64:P, :], CPY)
                                nc.vector.tensor_copy(stgb[64:P, :], ps[64:P, :])
                                nc.sync.dma_start(
                                    dstT[hB][0:64, sc * SH:(sc + 1) * SH],
                                    stg[64:P, :])
                                nc.sync.dma_start(
                                    dstA[hB][0:64, sc * SH:(sc + 1) * SH],
                                    stgb[64:P, :])
                    for sc4 in range(4):
                      vstr = [rhsp.tile([P, SH], bf16, name=f"vs{ic % 4}",
                                        tag=f"vs{ic % 4}") for ic in range(8)]
                      v8 = r8p.tile([P, 8 * SH], mybir.dt.int8,
                                    name="v8", tag="v8")
                      nc.sync.dma_start(
                          v8[:].rearrange("p (ic c) -> p ic c", ic=8),
                          ag_out3[2][D * sc4:D * (sc4 + 1), :].rearrange(
                              "(ic p) c -> p ic c", ic=8))
                      for ic in range(8):
                          nc.vector.tensor_copy(vstr[ic][:],
                                                v8[:, ic * SH:(ic + 1) * SH])
                      for qc in range(4):
                        kc = sc4 * 4 + qc
                        ps = psp3.tile([P, 256], f32, name="psv", tag="psv")
                        for ic in range(8):
                            nc.tensor.matmul(ps[:],
                                             vstr[ic][:, qc * P:(qc + 1) * P],
                                             wv_sb[ic][:],
                                             start=(ic == 0), stop=(ic == 7))
                        for h in range(4):
                            nc.scalar.activation(
                                vps[:, 260 * kc + 65 * h:260 * kc + 65 * h + 64],
                                ps[:, 64 * h:64 * h + 64], CPY)

                # ---- attention: singles-granularity software pipeline ----
                # per kc step of block i the PE runs [A(i+2,kc), B(i,kc),
                # AV(i,kc-4)]; exp fires per B single; pass A runs two blocks
                # ahead so the -M row lands a full block before B reads it;
                # the epilogue of block i-1 (ACT copy, PE transposes, DVE
                # division) is stitched into block i's steps 0-7 so the PE
                # never drains at a block boundary (keeps the 2.4GHz p-state).
                # PSUM: psA 2x[128,1024] + psB 2x[128,512] + psX + psT = 8.
                with tc.tile_pool(name="mpool", bufs=3) as mpool, \
                     tc.tile_pool(name="ptp", bufs=8) as ptp, \
                     tc.tile_pool(name="trp", bufs=2) as trp, \
                     tc.tile_pool(name="xtsb", bufs=2) as xtsb, \
                     tc.tile_pool(name="psA", bufs=4, space="PSUM") as psA, \
                     tc.tile_pool(name="psB", bufs=2, space="PSUM") as psB, \
                     tc.tile_pool(name="psX", bufs=1, space="PSUM") as psX, \
                     tc.tile_pool(name="psT", bufs=1, space="PSUM") as psT:

                    blocks = [(h, qb) for qb in range(4) for h in range(4)]

                    mp_tiles = {}
                    xps_tiles = {}
                    pa_cur = [None]
                    pT_lists = {}

                    def new_xps(blk):
                        xps_tiles[blk] = psX.tile([65, SH], f32,
                                                  name="xps", tag="xps")

                    def emit_A_step(blk, s):
                        """one bf16 QK single [128 q x 512 k] into a bf16
                        PSUM pair-tile half; the row-max estimate is built
                        with a bf16 tensor-tensor max tree (2x DVE mode; a
                        plain PSUM f32 reduce would pace the whole pipeline
                        below the 2.4GHz PE p-state). bf16 rounding of the
                        logits costs <~25 on the estimate; pass B's exact
                        logits-minus-M keeps exp args bounded either way."""
                        h, qb = blk
                        q0 = qb * SH
                        qc, kb = s // 4, s % 4
                        if kb == 0:
                            mp_tiles[blk + (qc,)] = mpool.tile(
                                [P, 4], f32, name="Mp", tag="Mp")
                        pa = psA.tile([P, SH], f32, name="pa", tag="pa")
                        nc.tensor.matmul(
                            pa[:],
                            qpA[h][:, q0 + qc * P:q0 + (qc + 1) * P],
                            kpA[h][:, kb * SH:(kb + 1) * SH],
                            start=True, stop=True)
                        Mp = mp_tiles[blk + (qc,)]
                        nc.vector.tensor_reduce(
                            Mp[:, kb:kb + 1], pa[:], axis=AXX, op=ALU.max)
                        if kb == 3:
                            ngm = mpool.tile([P, 1], f32r, name="ngm", tag="ngm")
                            nc.vector.tensor_reduce(ngm[:], Mp[:], axis=AXX,
                                                    op=ALU.max, negate=True)
                            nc.sync.dma_start(
                                qpT[h][64:65, q0 + qc * P:q0 + (qc + 1) * P],
                                ngm[:])
                            mp_tiles.pop(blk + (qc,))

                    def emit_B(blk, kc):
                        """one f32r QK single [128 k x 512 q] + its exp."""
                        h, qb = blk
                        q0 = qb * SH
                        pb = psB.tile([P, SH], f32, name="pb", tag="pb")
                        nc.tensor.matmul(
                            pb[:],
                            kpT[h][:, kc * P:(kc + 1) * P],
                            qpT[h][:, q0:q0 + SH],
                            start=True, stop=True)
                        pT = ptp.tile([P, SH], bf16, name="pT", tag="pT")
                        nc.scalar.activation(pT[:], pb[:], EXP, scale=Cq[:])
                        pT_lists[blk].append(pT)

                    def emit_AV(blk, kc):
                        h, qb = blk
                        nc.tensor.matmul(
                            xps_tiles[blk],
                            vps[:, 260 * kc + 65 * h:260 * kc + 65 * (h + 1)],
                            pT_lists[blk][kc],
                            start=(kc == 0), stop=(kc == 15))

                    def emit_epi_copy(blk):
                        """ACT copy drains xps -> SBUF, freeing the psX bank."""
                        xps = xps_tiles.pop(blk)
                        xt_s = xtsb.tile([65, SH], f32, name="xt_s", tag="xt_s")
                        nc.scalar.activation(xt_s[:], xps[:], CPY)
                        pT_lists.pop(blk)
                        return xt_s

                    def emit_epi_div(blk, xt_s, qc):
                        """transpose one 128-query chunk + divide by denom."""
                        h, qb = blk
                        ptx = psT.tile([P, 65], f32, name="ptx", tag="ptx")
                        nc.tensor.transpose(
                            ptx[:], xt_s[0:65, qc * P:(qc + 1) * P],
                            idf[0:65, 0:65])
                        rv = mpool.tile([P, 1], f32, name="rv", tag="rv")
                        rv0 = mpool.tile([P, 1], f32, name="rv0", tag="rv0")
                        nc.vector.reciprocal(rv0[:], ptx[:, 64:65])
                        nc.vector.tensor_tensor(rv[:], rv0[:], iCv[:],
                                                ALU.mult)
                        nc.vector.tensor_scalar(
                            xsb[qb * 4 + qc][:, 64 * h:64 * (h + 1)],
                            ptx[:, 0:64], rv[:], None, ALU.mult)

                    def emit_xag(qb):
                        for qc in range(4):
                            # scalar-triggered: keeps these off the sync
                            # stream, where pending -M DMAs would HOL-block
                            # them for more than a block
                            nc.scalar.dma_start(
                                xag_in[qb * SH + qc * P:
                                       qb * SH + (qc + 1) * P, :],
                                xsb[qb * 4 + qc][:])
                        nc.gpsimd.collective_compute(
                            "AllGather", ALU.bypass, replica_groups=groups4,
                            ins=[xag_in[qb * SH:(qb + 1) * SH, :].opt()],
                            outs=[xag_out4[qb].opt()])

                    # prologue: pass A for blocks 0 and 1
                    for s in range(16):
                        emit_A_step(blocks[0], s)
                    for s in range(16):
                        emit_A_step(blocks[1], s)

                    for i, blk in enumerate(blocks):
                        h, qb = blk
                        prv = blocks[i - 1] if i > 0 else None
                        nxt2 = blocks[i + 2] if i + 2 < len(blocks) else None
                        new_xps(blk)
                        pT_lists[blk] = []
                        xt_prev = [None]
                        for kc in range(16):
                            if prv is not None and kc < 4:
                                emit_AV(prv, 12 + kc)
                                if kc == 3:
                                    xt_prev[0] = emit_epi_copy(prv)
                            emit_B(blk, kc)
                            if nxt2 is not None:
                                emit_A_step(nxt2, kc)
                            if 4 <= kc < 8 and prv is not None:
                                emit_epi_div(prv, xt_prev[0], kc - 4)
                                if kc == 7 and prv[0] == 3:
                                    # previous qb group complete: ship its x
                                    emit_xag(prv[1])
                            if kc >= 4:
                                emit_AV(blk, kc - 4)
                        # spread deferred x-stats one tile per block so the
                        # DVE never pushes the block pace above the PE's;
                        # late blocks take a second tile to shorten the tail
                        if i >= 6:
                            emit_x_stat_tile(i - 6, mpool)

                    # flush: last block's AV tail + epilogue + its qb AG
                    lst = blocks[-1]
                    for kc in range(12, 16):
                        emit_AV(lst, kc)
                    xt_l = emit_epi_copy(lst)
                    for qc in range(4):
                        emit_epi_div(lst, xt_l, qc)
                    emit_xag(lst[1])

                if DEBUG:
                    with tc.tile_pool(name="dbgp", bufs=1) as dbgp:
                        nc.sync.dma_start(dbg_qpa, qpA[0][:, 0:256])
                        nc.sync.dma_start(dbg_kpa, kpA[0][:, 0:256])
                        nc.sync.dma_start(dbg_qpt, qpT[0][:, 0:256])
                        nc.sync.dma_start(dbg_kpt, kpT[0][:, 0:256])
                        nc.sync.dma_start(dbg_vps, vps[:, 0:260])
                        nc.sync.dma_start(dbg_xsb, xsb[0][:])

            # ====== stage 5 tail: last stats, AR2, quant, projection =====
            with tc.tile_pool(name="fpool", bufs=1) as fpool, \
                 tc.tile_pool(name="f2pool", bufs=3) as f2pool, \
                 tc.tile_pool(name="psf", bufs=1, space="PSUM") as psf:
                for rc in range(10, 16):
                    emit_x_stat_tile(rc, f2pool)
                xq0T = fpool.tile([P, 16 * D], bf16, name="xq0T", tag="xq0T")
                Am21 = fpool.tile([P, 1], f32, name="Am21", tag="Am21")
                nc.vector.tensor_reduce(Am21[:], Am2[:], axis=AXX, op=ALU.max)
                stage2 = fpool.tile([1, 8], f32, name="stage2", tag="stage2")
                nc.vector.memset(stage2[:], 0.0)
                tree_max(f2pool, psf, Am21, 1, stage2)
                nc.sync.dma_start(ar2_in[:], stage2[:])
                nc.gpsimd.collective_compute(
                    "AllReduce", ALU.max, replica_groups=groups8,
                    ins=[ar2_in.opt()], outs=[ar2_out.opt()])
                arow2 = fpool.tile([1, 8], f32, name="arow2", tag="arow2")
                nc.sync.dma_start(arow2[:], ar2_out[:])
                psb3 = psf.tile([P, 8], f32, name="psb3", tag="psb3")
                nc.tensor.matmul(psb3[:], ones1[:], arow2[:], start=True,
                                 stop=True)
                G2 = fpool.tile([P, 8], f32, name="G2", tag="G2")
                nc.vector.tensor_copy(G2[:], psb3[:])
                s128b = fpool.tile([P, 1], f32, name="s128b", tag="s128b")
                ig0, gl0 = rsqrt_nr(fpool, G2[:, 0:1], "f0")
                nc.vector.tensor_scalar(s128b[:], ig0[:], 128.0, None,
                                        ALU.mult)
                bb2 = fpool.tile([P, 8], f32, name="bb2", tag="bb2")
                psb4 = psf.tile([P, 8], f32, name="psb4", tag="psb3")
                nc.tensor.matmul(psb4[:], ones1[:], beta_sb[:], start=True,
                                 stop=True)
                nc.vector.tensor_copy(bb2[:], psb4[:])
                C0 = fpool.tile([P, 1], f32, name="C0", tag="C0")
                nc.vector.tensor_tensor(C0[:], gl0[:], bb2[:, 3:4], ALU.mult)
                nc.vector.tensor_scalar(C0[:], C0[:], 1.0 / 128.0, None,
                                        ALU.mult)
                # quant + flipped output projection, interleaved per 4-row
                # group so the PE matmuls overlap the next group's quant
                xq4 = xq0T[:].rearrange("p (rc ic c) -> p rc ic c",
                                        rc=16, ic=8, c=P)
                for r4 in range(4):
                    for rc in range(4 * r4, 4 * r4 + 4):
                        xf = f2pool.tile([P, D], bf16, name="xf2", tag="xf2")
                        qb = rc // 4
                        for j in range(4):
                            nc.sync.dma_start(
                                xf[:, 256 * j:256 * (j + 1)],
                                xag_out4[qb][SH * j + (rc % 4) * P:
                                             SH * j + (rc % 4 + 1) * P, :])
                        quant_tile(f2pool, psf, xf, rst2[rc], nmu2[rc],
                                   s128b, xq0T[:, rc * D:(rc + 1) * D],
                                   f"f{rc}")
                    for oh in range(2):
                        ps = psf.tile([P, SH], f32, name="pso", tag="pso",
                                      bufs=2)
                        for ic in range(8):
                            nc.tensor.matmul(
                                ps[:],
                                w0_sb[ic][:, oh * P:(oh + 1) * P],
                                xq4[:, 4 * r4:4 * (r4 + 1), ic, :],
                                start=(ic == 0), stop=(ic == 7))
                        yt = f2pool.tile([P, SH], f32, name="yt", tag="yt")
                        nc.scalar.activation(yt[:], ps[:], CPY, scale=C0[:])
                        nc.sync.dma_start(
                            y[oh * P:(oh + 1) * P, r4 * SH:(r4 + 1) * SH],
                            yt[:])

    nc.compile()
    return nc


def _prep_host(inputs):
    import ml_dtypes
    bf = ml_dtypes.bfloat16
    ws = [inputs["wq_w"], inputs["wk_w"], inputs["wv_w"], inputs["w0_w"]]
    signs = []
    betas = np.zeros((1, 8), np.float32)
    for i, w in enumerate(ws):
        w64 = np.asarray(w, np.float64)
        signs.append(np.sign(w64 - w64.mean()).astype(np.float32))
        betas[0, i] = np.abs(w64).mean()
    id_bf = np.eye(128, dtype=bf)
    id_f = np.eye(128, dtype=np.float32)
    ones_row = np.ones((1, S), np.float32)
    qf = np.asarray(inputs["q"], np.float32).reshape(2 * S, D)
    kf = np.asarray(inputs["k"], np.float32).reshape(2 * S, D)
    vf = np.asarray(inputs["v"], np.float32).reshape(2 * S, D)
    in_maps = []
    for c in range(8):
        b, g = c // 4, c % 4
        r0 = b * S + g * SH
        m = {
            "x0": np.ascontiguousarray(qf[r0:r0 + SH]),
            "x1": np.ascontiguousarray(kf[r0:r0 + SH]),
            "x2": np.ascontiguousarray(vf[r0:r0 + SH]),
            "wo": np.ascontiguousarray(
                signs[3].T[:, 256 * g:256 * (g + 1)]).astype(bf),
            "idbf": id_bf, "idf": id_f,
            "onesrow": ones_row, "betas": betas,
        }
        for t in range(3):
            m[f"w{t}"] = np.ascontiguousarray(
                signs[t].T[:, 256 * g:256 * (g + 1)]).astype(bf)
        in_maps.append(m)
    return in_maps


def _run(inputs, trace=False):
    global _COMPILED
    from concourse import bass_utils
    if _COMPILED is None:
        _COMPILED = _build()
    nc = _COMPILED
    in_maps = _prep_host(inputs)
    res = bass_utils.run_bass_kernel_spmd(nc, in_maps, core_ids=list(range(8)),
                                          trace=trace)
    out = np.zeros((B, S, D), np.float32)
    for c in range(8):
        b, g = c // 4, c % 4
        out[b, :, 256 * g:256 * (g + 1)] = res.results[c]["y"].T
    return out, res


def kernel(**inputs):
    mask = np.asarray(inputs["mask"])
    if not (mask == 1).all():
        return _numpy_fallback(**inputs)
    out, _ = _run(inputs, trace=False)
    return out


def _numpy_fallback(q, k, v, mask, wq_w, wk_w, wv_w, w0_w):
    f = np.float32

    def ln(x):
        mu = x.mean(-1, keepdims=True, dtype=f)
        var = np.mean((x - mu) ** 2, -1, keepdims=True, dtype=f)
        return ((x - mu) / np.sqrt(var + f(LN_EPS))).astype(f)

    def bitlin(x, w):
        xn = ln(np.asarray(x, f))
        mx = np.abs(xn).max()
        xq = np.round(xn * (f(128.0) / mx)).astype(f)
        xq = (np.mod(xq + 128.0, 256.0) - 128.0).astype(f)
        wq = np.sign(w - w.mean(dtype=f)).astype(f)
        beta = np.abs(w).mean(dtype=f)
        return ((xq @ wq.T) * f(mx / 128 * beta)).astype(f)

    qp = bitlin(q, wq_w).reshape(B, S, H, DK).transpose(0, 2, 1, 3)
    kp = bitlin(k, wk_w).reshape(B, S, H, DK).transpose(0, 2, 1, 3)
    vp = bitlin(v, wv_w).reshape(B, S, H, DK).transpose(0, 2, 1, 3)
    out = np.zeros((B, H, S, DK), f)
    mask = np.asarray(mask)
    for b in range(B):
        for h in range(H):
            att = (qp[b, h] @ kp[b, h].T) / f(np.sqrt(DK))
            att = np.where(mask[b] == 0, f(-1e9), att).astype(f)
            att = att - att.max(-1, keepdims=True)
            e = np.exp(att)
            p = e / e.sum(-1, keepdims=True)
            out[b, h] = p @ vp[b, h]
    x = out.transpose(0, 2, 1, 3).reshape(B, S, H * DK)
    return bitlin(x, w0_w)

